# revision 56
# baseline (speedup 1.0000x reference)
"""Trainium2 Bass kernel for nn_Block_87351044866235 (sparse_attention).

Data-parallel over batch: 8 samples -> 8 NeuronCores. Channel-major
layout [C, H*W] on chip; depthwise convs as diagonal fp32r matmuls on
TensorE; 1x1 convs as fp32r matmuls; LN stats via ones-matmuls; q/k gram
via hi/lo bf16 split + DMA-xbar transposes; dynamic-k gate mean via a
scalar AllReduce.
"""
import sys, os

for _p in ("/opt/trn_rl_repo", "/root/.axon_site/_ro/trn_rl_repo"):
    if os.path.isdir(_p) and _p not in sys.path:
        sys.path.append(_p)

import numpy as np
import concourse.bass as bass
import concourse.bacc as bacc
import concourse.tile as tile
from concourse import mybir
from concourse import bass_utils

try:
    from concourse import tile_utils as _tu
    _tu.max_sbuf_usage = 208 * 1024
except Exception:
    pass

dt = mybir.dt
Alu = mybir.AluOpType
Act = mybir.ActivationFunctionType
AX = mybir.AxisListType.X

EMBED, PDIM, HEADS, HID = 192, 96, 8, 256
CPH = PDIM // HEADS  # 12
SLOP = 8
RC = 3    # conv output rows per chunk
BR = 12   # rows per band


F32, F32R, BF16 = dt.float32, dt.float32r, dt.bfloat16


def _ceil(a, b):
    return (a + b - 1) // b


# ----------------------------------------------------------------------------
# host-side weight prep: everything 2D [partitions, free]
# ----------------------------------------------------------------------------

def _prep_weights(p):
    w = {}
    f32r = lambda a: (np.ascontiguousarray(a, np.float32), F32R)
    f32 = lambda a: (np.ascontiguousarray(a, np.float32), F32)
    eps_bn = 1e-5

    w["ident"] = f32(np.eye(128, dtype=np.float32))
    w["identr"] = f32r(np.eye(128, dtype=np.float32))

    # pos depthwise diag: [96, (t*2+cg)*96]
    pw = p["pos_w"][:, 0]  # [192,3,3]
    pos_d = np.zeros((96, 18 * 96), np.float32)
    for t in range(9):
        dy, dx = t // 3 - 1, t % 3 - 1
        for cg in range(2):
            pos_d[:, (t * 2 + cg) * 96:(t * 2 + cg + 1) * 96] = \
                np.diag(pw[cg * 96:(cg + 1) * 96, dy + 1, dx + 1])
    w["pos_diag"] = f32r(pos_d)
    w["pos_b"] = f32(p["pos_b"].reshape(2, 96).T)  # [96, 2]

    g1v, b1v = p["ln1_g"], p["ln1_b"]
    qw = p["qkv_w"][:, :, 0, 0]  # [288, 96]
    qw_eff = qw * g1v[None, :96]
    w["qkv_wT"] = f32r(np.concatenate(
        [qw_eff[j * 96:(j + 1) * 96].T for j in range(3)], axis=1))  # [96, 3*96]
    w["qkv_bias"] = f32((qw @ b1v[:96]).reshape(3, 96).T)  # [96, 3]

    qdw = p["qkv_dw_w"][:, 0]  # [288,3,3]
    qdw_d = np.zeros((96, 27 * 96), np.float32)
    for t in range(9):
        dy, dx = t // 3 - 1, t % 3 - 1
        for j in range(3):
            qdw_d[:, (t * 3 + j) * 96:(t * 3 + j + 1) * 96] = \
                np.diag(qdw[j * 96:(j + 1) * 96, dy + 1, dx + 1])
    w["qdw_diag"] = f32r(qdw_d)

    gw1 = p["gate_w1"][:, :, 0, 0]  # [96, 192]
    gw1_eff = gw1 * g1v[None, :]
    w["gate_w1T"] = f32r(np.concatenate(
        [gw1_eff[:, cg * 96:(cg + 1) * 96].T for cg in range(2)], axis=1))  # [96, 192]
    w["gate_b1"] = f32((p["gate_b1"] + gw1 @ b1v).reshape(96, 1))
    w["gate_w2T"] = f32r(p["gate_w2"][:, :, 0, 0].T.copy())  # [96,1]
    w["gate_b2"] = f32(p["gate_b2"].reshape(1, 1))

    pj = p["proj_w"][:, :, 0, 0]
    pj1, pj2 = pj[:, :96], pj[:, 96:] * g1v[None, 96:]
    w["proj1T"] = f32r(np.concatenate(
        [pj1[cg * 96:(cg + 1) * 96].T for cg in range(2)], axis=1))  # [96, 192]
    w["proj2T"] = f32r(np.concatenate(
        [pj2[cg * 96:(cg + 1) * 96].T for cg in range(2)], axis=1))
    w["proj_bias"] = f32((pj[:, 96:] @ b1v[96:]).reshape(2, 96).T)  # [96, 2]

    attn_scale = float(p["attn1"][0] + p["attn2"][0] + p["attn3"][0] + p["attn4"][0])
    w["_attn_scale"] = (attn_scale, None)
    w["tempvec"] = f32(np.repeat(p["temperature"].reshape(HEADS), CPH).reshape(96, 1))

    g2v, b2v = p["ln2_g"], p["ln2_b"]
    f1 = p["fc1_w"][:, :, 0, 0]  # [256, 192]
    f1_eff = f1 * g2v[None, :]
    # channel groups 64|128|64 so each group's gelu lands in one strided op:
    # g0 -> vg0 (ch 0:64), g1 -> v0t1 (ch 64:192), g2 -> v0t2 (ch 192:256)
    fc1 = np.zeros((96, 512), np.float32)
    fc1[:, 0:64] = f1_eff[0:64, 0:96].T
    fc1[:, 64:128] = f1_eff[0:64, 96:192].T
    fc1[:, 128:256] = f1_eff[64:192, 0:96].T
    fc1[:, 256:384] = f1_eff[64:192, 96:192].T
    fc1[:, 384:448] = f1_eff[192:256, 0:96].T
    fc1[:, 448:512] = f1_eff[192:256, 96:192].T
    w["fc1T"] = f32r(fc1)
    fb = f1 @ b2v
    fbias = np.zeros((128, 2), np.float32)
    fbias[0:64, 0] = fb[0:64]
    fbias[64:128, 0] = fb[192:256]
    fbias[:, 1] = fb[64:192]
    w["fc1_bias"] = f32(fbias)

    s1 = p["bn1_g"] / np.sqrt(p["bn1_v"] + eps_bn)
    t1 = p["bn1_b"] - p["bn1_m"] * s1
    s2 = p["bn2_g"] / np.sqrt(p["bn2_v"] + eps_bn)
    t2 = p["bn2_b"] - p["bn2_m"] * s2
    s3 = p["bn3_g"] / np.sqrt(p["bn3_v"] + eps_bn)
    t3 = p["bn3_b"] - p["bn3_m"] * s3

    dw1w, dw2w, dw3w = p["dw1_w"][:, 0], p["dw2_w"][:, 0], p["dw3_w"][:, 0]
    dw1b, dw2b, dw3b = p["dw1_b"], p["dw2_b"], p["dw3_b"]
    s1g = [s1[i * 64:(i + 1) * 64] for i in range(4)]
    t1g = [t1[i * 64:(i + 1) * 64] for i in range(4)]

    def pair_tap_diag(t):
        dy, dx = t // 5 - 2, t % 5 - 2
        v = np.zeros(128, np.float32)
        d2 = dw2w[:, dy + 2, dx + 2] * s1g[2]
        if dy == 0 and dx == 0:
            d2 = d2 + s1g[2]
        v[64:] = d2
        if -1 <= dy <= 1 and -1 <= dx <= 1:
            d1 = dw1w[:, dy + 1, dx + 1] * s1g[1]
            if dy == 0 and dx == 0:
                d1 = d1 + s1g[1]
            v[:64] = d1
        return v

    pair_d = np.zeros((128, 25 * 128), np.float32)
    for t in range(25):
        pair_d[:, t * 128:(t + 1) * 128] = np.diag(pair_tap_diag(t))
    w["pair_diag"] = f32r(pair_d)
    bc1 = t1g[1] * dw1w.sum((1, 2)) + dw1b + t1g[1]
    bc2 = t1g[2] * dw2w.sum((1, 2)) + dw2b + t1g[2]
    w["pair_bias"] = f32(np.concatenate([bc1, bc2]).reshape(128, 1))

    # rows 64:128 of v0t2 hold the same data stored shifted +1, so a read at
    # AP offset (dy, dxa) yields tap (dy, dxa-1) for those rows.
    dw3_passes = []
    for dy in range(-3, 4):
        for dxa in (-2, 0, 2):
            dw3_passes.append((dy, dxa, True))
        dw3_passes.append((dy, 3, False))
    dw3_d = np.zeros((128, len(dw3_passes) * 64), np.float32)
    for i, (dy, dxa, hasb) in enumerate(dw3_passes):
        wa = dw3w[:, dy + 3, dxa + 3] * s1g[3]
        if dy == 0 and dxa == 0:
            wa = wa + s1g[3]
        dw3_d[:64, i * 64:(i + 1) * 64] = np.diag(wa)
        if hasb:
            wb = dw3w[:, dy + 3, dxa - 1 + 3] * s1g[3]
            if dy == 0 and dxa - 1 == 0:
                wb = wb + s1g[3]
            dw3_d[64:, i * 64:(i + 1) * 64] = np.diag(wb)
    w["dw3_diag"] = f32r(dw3_d)
    w["_dw3_passes"] = (dw3_passes, None)
    w["dw3_bias"] = f32((t1g[3] * dw3w.sum((1, 2)) + dw3b + t1g[3]).reshape(64, 1))

    d0w, d0b = p["dw0_w"][:, 0, 0, 0], p["dw0_b"]
    w["g0_scale"] = f32(((d0w + 1.0) * s1g[0]).reshape(64, 1))
    w["g0_bias"] = f32(((d0w + 1.0) * t1g[0] + d0b).reshape(64, 1))

    f2 = p["fc2_w"][:, :, 0, 0]  # [192, 256]
    f2a = f2 * s2[None, :]
    f2b = f2 * (t2 * s1)[None, :]
    cstv = f2 @ (t2 * t1)
    fc2a = np.zeros((128, 4 * 96), np.float32)
    for cg in range(2):
        for kg in range(2):
            fc2a[:, (cg * 2 + kg) * 96:(cg * 2 + kg + 1) * 96] = \
                f2a[cg * 96:(cg + 1) * 96, kg * 128:(kg + 1) * 128].T
    w["fc2aT"] = f32r(fc2a)
    w["fc2bT_g0"] = f32r(np.concatenate(
        [f2b[cg * 96:(cg + 1) * 96, 0:64].T for cg in range(2)], axis=1))    # [64, 192]
    w["fc2bT_g12"] = f32r(np.concatenate(
        [f2b[cg * 96:(cg + 1) * 96, 64:192].T for cg in range(2)], axis=1))  # [128, 192]
    w["fc2bT_g3"] = f32r(np.concatenate(
        [f2b[cg * 96:(cg + 1) * 96, 192:256].T for cg in range(2)], axis=1))  # [64, 192]
    w["s3v"] = f32(np.stack([s3[:96], s3[96:]], axis=1))          # [96, 2]
    w["out_bias"] = f32(np.stack([(s3 * 0 + t3 + s3 * cstv * 0)[:96], (t3)[96:]], axis=1))
    # careful: out = s3*(psum + cst) + t3 + xc' ; psum excludes cst, so bias = s3*cst + t3
    ob = s3 * cstv + t3
    w["out_bias"] = f32(np.stack([ob[:96], ob[96:]], axis=1))     # [96, 2]

    sg = np.where(s1 == 0, 1.0, s1)
    padv = -t1 / sg
    w["padv1"] = f32(np.concatenate([padv[64:128], padv[128:192]]).reshape(128, 1))
    w["padv2"] = f32(np.concatenate([padv[192:256], padv[192:256]]).reshape(128, 1))
    w["s1a"] = f32(s1[:128].reshape(128, 1))
    w["s1b"] = f32(s1[128:].reshape(128, 1))
    w["t1a"] = f32(t1[:128].reshape(128, 1))
    w["t1b"] = f32(t1[128:].reshape(128, 1))

    w["ones_st"] = f32r(np.full((96, 128), 1.0 / EMBED, np.float32))
    w["epsv"] = f32(np.full((128, 1), 1e-6, np.float32))
    vm = np.zeros((96, 96), np.float32)
    for h in range(HEADS):
        vm[h * CPH:(h + 1) * CPH, h * CPH:(h + 1) * CPH] = 1.0
    w["vmask"] = f32(vm)
    return w


WSPEC = {
    "ident": ([128, 128], F32), "identr": ([128, 128], F32R),
    "pos_diag": ([96, 18 * 96], F32R),
    "pos_b": ([96, 2], F32), "qkv_wT": ([96, 3 * 96], F32R),
    "qkv_bias": ([96, 3], F32), "qdw_diag": ([96, 27 * 96], F32R),
    "gate_w1T": ([96, 192], F32R), "gate_b1": ([96, 1], F32),
    "gate_w2T": ([96, 1], F32R), "gate_b2": ([1, 1], F32),
    "proj1T": ([96, 192], F32R), "proj2T": ([96, 192], F32R),
    "proj_bias": ([96, 2], F32), "tempvec": ([96, 1], F32),
    "fc1T": ([96, 4 * 128], F32R), "fc1_bias": ([128, 2], F32),
    "pair_diag": ([128, 25 * 128], F32R), "pair_bias": ([128, 1], F32),
    "dw3_diag": ([128, 28 * 64], F32R), "dw3_bias": ([64, 1], F32),
    "g0_scale": ([64, 1], F32), "g0_bias": ([64, 1], F32),
    "fc2aT": ([128, 4 * 96], F32R), "fc2bT_g0": ([64, 192], F32R),
    "fc2bT_g12": ([128, 192], F32R), "fc2bT_g3": ([64, 192], F32R),
    "s3v": ([96, 2], F32), "out_bias": ([96, 2], F32),
    "padv1": ([128, 1], F32),
    "padv2": ([128, 1], F32),
    "s1a": ([128, 1], F32), "s1b": ([128, 1], F32),
    "t1a": ([128, 1], F32), "t1b": ([128, 1], F32),
    "ones_st": ([96, 128], F32R),
    "epsv": ([128, 1], F32),
    "vmask": ([96, 96], F32),
}


# ----------------------------------------------------------------------------
# device kernel
# ----------------------------------------------------------------------------

def build(nc, H, W, n_cores, attn_scale, dw3_passes):
    S = H * W
    Wp1 = W + 2
    P1B = (BR + 2) * Wp1 + 2 * SLOP   # band buffer (pad1)
    Wp3, Hp3 = W + 6, H + 6
    P3 = Hp3 * Wp3 + 2 * SLOP
    NCH = _ceil(H, RC)
    NB = _ceil(H, BR)
    NSC = _ceil(S, 512)
    GCH = 512 // W                    # gate chunk rows (512 cols)
    NGC_PER_BAND = _ceil(BR, GCH)

    x_t = nc.dram_tensor("x", [H, W, EMBED], F32, kind="ExternalInput")
    out_t = nc.dram_tensor("out", [S, EMBED], F32, kind="ExternalOutput")
    wt = {k: nc.dram_tensor("w_" + k, shp, d, kind="ExternalInput")
          for k, (shp, d) in WSPEC.items()}

    def pd3(r):
        return SLOP + r * Wp3

    with tile.TileContext(nc) as tc:
        C_ONLY_W = ['fc1T', 'fc1_bias', 'pair_diag', 'pair_bias', 'dw3_diag', 'dw3_bias', 'g0_scale', 'g0_bias', 'fc2aT', 'fc2bT_g0', 'fc2bT_g12', 'fc2bT_g3', 's3v', 'out_bias', 's1a', 's1b', 't1a', 't1b', 'padv1', 'padv2']
        with (
            tc.tile_pool(name="dram", bufs=1, space="DRAM") as dram,
            tc.tile_pool(name="persist", bufs=1) as pers,
        ):
            ws = {}

            def _load_w(pool, names):
                for k in names:
                    shp, d = WSPEC[k]
                    tl = pool.tile(shp, d, tag="w_" + k, name="w_" + k)
                    nc.sync.dma_start(out=tl[:], in_=wt[k][:])
                    ws[k] = tl


            yn1_sp = dram.tile([96, S], F32R)
            yn2_sp = dram.tile([96, S], F32R)
            xc_sp = [dram.tile([96, S], F32R, name=f"xc_sp{i}") for i in range(2)]
            v_sp = dram.tile([96, S], F32R)
            xcp_sp = [dram.tile([96, S], F32R, name=f"xcp_sp{i}") for i in range(2)]
            ynn_sp = [dram.tile([96, S], F32R, name=f"ynn_sp{i}") for i in range(2)]
            vg0_sp = dram.tile([64, P3], F32R)
            ug0_sp = dram.tile([64, P3], F32R)
            dbg_sp = {nm: dram.tile([128, S], F32, name="dbg_" + nm)
                      for nm in ("uga", "ugb", "vba", "vbb", "z1a", "z1b")} \
                if getattr(build, "DEBUG", False) else None
            cc_in = dram.tile([1, 1], F32)
            cc_out = dram.tile([1, 1], F32)

            gsum = pers.tile([1, NB * NGC_PER_BAND + 8], F32)
            nc.vector.memset(gsum[:], 0.0)
            dynk = pers.tile([96, 1], F32)
            probsT = pers.tile([96, 96], F32R)
            _load_w(pers, ["ident", "identr"])
            ident = ws["ident"]
            identr = ws["identr"]

            # ================= PHASE A =================
            _wpab_cm = tc.tile_pool(name="wpAB", bufs=1)
            wpab = _wpab_cm.__enter__()
            _load_w(wpab, [k for k in WSPEC if k not in C_ONLY_W and k != "ident"])
            with (
                tc.tile_pool(name="pa_band", bufs=2) as pab,
                tc.tile_pool(name="pa_rot", bufs=3) as par,
                tc.tile_pool(name="pa_ps", bufs=2, space="PSUM") as paps,
                tc.tile_pool(name="pa_ps2", bufs=2, space="PSUM") as paps2,
            ):
                for b in range(NB):
                    r0, r1 = b * BR, min((b + 1) * BR, H)
                    xband = [pab.tile([96, P1B], F32R, tag=f"xb{cg}", name=f"xb{cg}") for cg in range(2)]
                    for cg in range(2):
                        nc.vector.memset(xband[cg][:].bitcast(F32), 0.0)
                    for rr in range(max(r0 - 1, 0), min(r1 + 1, H)):
                        xrow = par.tile([W, EMBED], F32, tag="xrow")
                        nc.sync.dma_start(out=xrow[:], in_=x_t[rr])
                        boff = SLOP + (rr - (r0 - 1)) * Wp1 + 1
                        for cg in range(2):
                            tps = paps2.tile([96, W], F32, tag="tps")
                            nc.tensor.transpose(tps[:], xrow[:, cg * 96:(cg + 1) * 96],
                                                ident[:W, :W])
                            nc.scalar.copy(xband[cg][:, boff:boff + W], tps[:])
                    for c0 in range(r0, r1, RC):
                        nr_c = min(RC, H - c0)
                        N = nr_c * Wp1
                        NN = nr_c * W
                        sb0 = SLOP + (c0 - r0 + 1) * Wp1
                        xc_ch = [par.tile([96, RC * W], F32R, tag=f"xc{cg}", name=f"xc{cg}") for cg in range(2)]
                        xsq = [par.tile([96, RC * W], F32R, tag=f"xq{cg}", name=f"xq{cg}") for cg in range(2)]
                        for cg in range(2):
                            ps = paps.tile([96, RC * Wp1], F32, tag="posps")
                            for t in range(9):
                                dy, dx = t // 3 - 1, t % 3 - 1
                                o = sb0 + dy * Wp1 + dx
                                nc.tensor.matmul(
                                    ps[:, :N],
                                    ws["pos_diag"][:, (t * 2 + cg) * 96:(t * 2 + cg + 1) * 96],
                                    xband[cg][:, o:o + N],
                                    start=(t == 0), stop=(t == 8))
                            ps_int = ps[:, :N].rearrange("p (r w) -> p r w", w=Wp1)[:, :, 1:1 + W]
                            xb_int = xband[cg][:, sb0:sb0 + N] \
                                .rearrange("p (r w) -> p r w", w=Wp1)[:, :, 1:1 + W]
                            xcv = xc_ch[cg][:, :NN].rearrange("p (r w) -> p r w", w=W)
                            nc.vector.scalar_tensor_tensor(
                                out=xcv, in0=ps_int, scalar=ws["pos_b"][:, cg:cg + 1],
                                in1=xb_int, op0=Alu.add, op1=Alu.add)
                            nc.scalar.square(xsq[cg][:, :NN], xc_ch[cg][:, :NN])
                        mu_ps = paps.tile([128, RC * W], F32, tag="mups")
                        m2_ps = paps.tile([128, RC * W], F32, tag="m2ps")
                        for cg in range(2):
                            nc.tensor.matmul(mu_ps[:, :NN], ws["ones_st"], xc_ch[cg][:, :NN],
                                             start=(cg == 0), stop=(cg == 1))
                            nc.tensor.matmul(m2_ps[:, :NN], ws["ones_st"], xsq[cg][:, :NN],
                                             start=(cg == 0), stop=(cg == 1))
                        musq = par.tile([128, RC * W], F32, tag="musq")
                        nc.scalar.square(musq[:, :NN], mu_ps[:, :NN])
                        var = par.tile([128, RC * W], F32, tag="var")
                        nc.vector.tensor_tensor(out=var[:, :NN], in0=m2_ps[:, :NN],
                                                in1=musq[:, :NN], op=Alu.subtract)
                        sd = par.tile([128, RC * W], F32, tag="sd")
                        nc.scalar.activation(sd[:, :NN], var[:, :NN], Act.Sqrt, bias=ws["epsv"])
                        rstd = par.tile([128, RC * W], F32, tag="rstd")
                        nc.vector.reciprocal_approx_fast(rstd[:, :NN], sd[:, :NN])
                        for cg in range(2):
                            tdf = par.tile([96, RC * W], F32, tag=f"td{cg}")
                            nc.vector.tensor_tensor(out=tdf[:, :NN], in0=xc_ch[cg][:, :NN],
                                                    in1=mu_ps[:96, :NN], op=Alu.subtract)
                            ynch = par.tile([96, RC * W], F32R, tag=f"yn{cg}")
                            nc.vector.tensor_tensor(out=ynch[:, :NN], in0=tdf[:, :NN],
                                                    in1=rstd[:96, :NN], op=Alu.mult)
                            sp = yn1_sp if cg == 0 else yn2_sp
                            nc.sync.dma_start(out=sp[:, c0 * W:c0 * W + NN],
                                              in_=ynch[:, :NN])
                            nc.sync.dma_start(out=xc_sp[cg][:, c0 * W:c0 * W + NN],
                                              in_=xc_ch[cg][:, :NN])

            # ================= PHASE B =================
            with (
                tc.tile_pool(name="pb_band", bufs=2) as pbb,
                tc.tile_pool(name="pb_rot", bufs=3) as pbr,
                tc.tile_pool(name="gram_ps", bufs=1, space="PSUM") as gpsp,
            ):
                # one bank: [0:96]=q.q, [96:192]=q.k, [192:288]=k.k
                g1_ps = gpsp.tile([96, 288], F32)
                with (
                    tc.tile_pool(name="pb_psg", bufs=1, space="PSUM") as pbpsg,
                    tc.tile_pool(name="pb_ps", bufs=2, space="PSUM") as pbps,
                ):
                    for b in range(NB):
                        r0, r1 = b * BR, min((b + 1) * BR, H)
                        ylo, yhi = max(r0 - 1, 0), min(r1 + 1, H)
                        ynb = [pbb.tile([96, (BR + 2) * W], F32R, tag=f"ynb{cg}", name=f"ynb{cg}")
                               for cg in range(2)]
                        for cg in range(2):
                            sp = yn1_sp if cg == 0 else yn2_sp
                            nc.sync.dma_start(
                                out=ynb[cg][:, (ylo - r0 + 1) * W:(yhi - r0 + 1) * W],
                                in_=sp[:, ylo * W:yhi * W])
                        # gate (512-col chunks over rows [r0, r1))
                        for gi in range(NGC_PER_BAND):
                            gr0 = r0 + gi * GCH
                            if gr0 >= r1:
                                break
                            ngr = min(GCH, r1 - gr0)
                            NG = ngr * W
                            yo = (gr0 - r0 + 1) * W
                            gps = pbpsg.tile([96, 512], F32, tag="gps")
                            for cg in range(2):
                                nc.tensor.matmul(gps[:, :NG],
                                                 ws["gate_w1T"][:, cg * 96:(cg + 1) * 96],
                                                 ynb[cg][:, yo:yo + NG],
                                                 start=(cg == 0), stop=(cg == 1))
                            g1s = pbr.tile([96, 512], F32R, tag="g1s")
                            nc.scalar.activation(g1s[:, :NG], gps[:, :NG], Act.Relu,
                                                 bias=ws["gate_b1"])
                            g2ps = pbpsg.tile([1, 512], F32, tag="g2ps")
                            nc.tensor.matmul(g2ps[:, :NG], ws["gate_w2T"], g1s[:, :NG],
                                             start=True, stop=True)
                            sgt = pbr.tile([1, 512], F32, tag="sgt")
                            idx = b * NGC_PER_BAND + gi
                            nc.scalar.activation(sgt[:, :NG], g2ps[:, :NG], Act.Sigmoid,
                                                 bias=ws["gate_b2"],
                                                 accum_out=gsum[0:1, idx:idx + 1])
                        # qkv0 band
                        qkv0 = [pbb.tile([96, P1B], F32R, tag=f"qk0{j}", name=f"qk0{j}") for j in range(3)]
                        for j in range(3):
                            nc.vector.memset(qkv0[j][:].bitcast(F32), 0.0)
                        for rr in range(ylo, yhi, 4):
                            nrw = min(4, yhi - rr)
                            NQ = nrw * W
                            for j in range(3):
                                qps = pbps.tile([96, 4 * W], F32, tag="qps", bufs=1)
                                nc.tensor.matmul(qps[:, :NQ],
                                                 ws["qkv_wT"][:, j * 96:(j + 1) * 96],
                                                 ynb[0][:, (rr - r0 + 1) * W:(rr - r0 + 1) * W + NQ],
                                                 start=True, stop=True)
                                dst = SLOP + (rr - r0 + 1) * Wp1 + 1
                                dview = qkv0[j][:, dst:dst + nrw * Wp1] \
                                    .rearrange("p (r w) -> p r w", w=Wp1)[:, :, 0:W]
                                nc.scalar.activation(
                                    dview, qps[:, :NQ].rearrange("p (r w) -> p r w", w=W),
                                    Act.Identity, bias=ws["qkv_bias"][:, j:j + 1])
                        # depthwise, then per-row TensorE transpose + fp32 gram
                        for c0 in range(r0, r1, RC):
                            nr_c = min(RC, H - c0)
                            N = nr_c * Wp1
                            NN = nr_c * W
                            sb0 = SLOP + (c0 - r0 + 1) * Wp1
                            qk_ch = {}
                            for j in range(3):
                                ps = pbps.tile([96, RC * Wp1], F32, tag="dwps")
                                for t in range(9):
                                    dy, dx = t // 3 - 1, t % 3 - 1
                                    o = sb0 + dy * Wp1 + dx
                                    nc.tensor.matmul(
                                        ps[:, :N],
                                        ws["qdw_diag"][:, (t * 3 + j) * 96:(t * 3 + j + 1) * 96],
                                        qkv0[j][:, o:o + N],
                                        start=(t == 0), stop=(t == 8))
                                ps_int = ps[:, :N].rearrange("p (r w) -> p r w", w=Wp1)[:, :, 1:1 + W]
                                ch = pbr.tile([96, RC * W], F32R, tag=f"qkv_ch{j}")
                                nc.scalar.copy(
                                    ch[:, :NN].rearrange("p (r w) -> p r w", w=W), ps_int)
                                if j == 2:
                                    nc.sync.dma_start(out=v_sp[:, c0 * W:c0 * W + NN],
                                                      in_=ch[:, :NN])
                                else:
                                    qk_ch[j] = ch
                            for rr in range(c0, c0 + nr_c):
                                rl = (rr - c0) * W
                                tqk_ps = pbps.tile([W, 192], F32, tag="tqk")
                                nc.tensor.transpose(
                                    tqk_ps[:, 0:96],
                                    qk_ch[0][:, rl:rl + W].bitcast(F32),
                                    ident[:96, :96])
                                nc.tensor.transpose(
                                    tqk_ps[:, 96:192],
                                    qk_ch[1][:, rl:rl + W].bitcast(F32),
                                    ident[:96, :96])
                                qkT = pbr.tile([W, 192], F32, tag="qkT")
                                nc.scalar.copy(qkT[:], tqk_ps[:])
                                nc.tensor.matmul(g1_ps[:, 0:192], qkT[:, 0:96],
                                                 qkT[:, 0:192],
                                                 start=(rr == 0), stop=(rr == H - 1))
                                nc.tensor.matmul(g1_ps[:, 192:288], qkT[:, 96:192],
                                                 qkT[:, 96:192],
                                                 start=(rr == 0), stop=(rr == H - 1))

                # ---- gate mean -> AllReduce -> dynk ----
                gred = pers.tile([1, 1], F32)
                nc.vector.reduce_sum(gred[:], gsum[0:1, 0:NB * NGC_PER_BAND], axis=AX)
                gsc = pers.tile([1, 1], F32)
                nc.vector.tensor_scalar_mul(gsc[:], gred[:], float(CPH) / (n_cores * S))
                nc.sync.dma_start(out=cc_in[:], in_=gsc[:])
                nc.gpsimd.collective_compute(
                    "AllReduce", Alu.add, replica_groups=[list(range(n_cores))],
                    ins=[cc_in.opt()], outs=[cc_out.opt()])
                nc.sync.dma_start(out=dynk[:], in_=cc_out[:].partition_broadcast(96))

                # ---- attn block ----
                with (
                    tc.tile_pool(name="at_ps", bufs=2, space="PSUM") as atps,
                    tc.tile_pool(name="at_sb", bufs=1) as ab,
                ):
                    g1sb = ab.tile([96, 288], F32)
                    nc.scalar.copy(g1sb[:], g1_ps[:])
                    gqk = g1sb[:, 96:192]
                    idm = ident[:96, :96]
                    tq = ab.tile([96, 96], F32)
                    nc.vector.tensor_tensor(out=tq[:], in0=g1sb[:, 0:96], in1=idm, op=Alu.mult)
                    nq2 = ab.tile([96, 1], F32)
                    nc.vector.reduce_sum(nq2[:], tq[:], axis=AX)
                    tk = ab.tile([96, 96], F32)
                    nc.vector.tensor_tensor(out=tk[:], in0=g1sb[:, 192:288], in1=idm,
                                            op=Alu.mult)
                    nk2 = ab.tile([96, 1], F32)
                    nc.vector.reduce_sum(nk2[:], tk[:], axis=AX)

                    def rsqrt_clamped(nm, src):
                        sq = ab.tile([96, 1], F32, tag=nm + "sq")
                        nc.scalar.sqrt(sq[:], src[:])
                        cl = ab.tile([96, 1], F32, tag=nm + "cl")
                        nc.vector.tensor_scalar_max(cl[:], sq[:], 1e-12)
                        rvv = ab.tile([96, 1], F32, tag=nm)
                        nc.vector.reciprocal_approx_fast(rvv[:], cl[:])
                        return rvv

                    rq = rsqrt_clamped("rq", nq2)
                    rk = rsqrt_clamped("rk", nk2)
                    rqt = ab.tile([96, 1], F32)
                    nc.vector.tensor_tensor(out=rqt[:], in0=rq[:], in1=ws["tempvec"][:],
                                            op=Alu.mult)
                    asr = ab.tile([96, 96], F32)
                    nc.vector.tensor_scalar_mul(asr[:], gqk, rqt[:])
                    as_ps = atps.tile([96, 96], F32, tag="atp")
                    nc.tensor.transpose(as_ps[:], asr[:], ident[:96, :96])
                    ast = ab.tile([96, 96], F32)
                    nc.vector.tensor_scalar_mul(ast[:], as_ps[:], rk[:])
                    as2_ps = atps.tile([96, 96], F32, tag="atp")
                    nc.tensor.transpose(as2_ps[:], ast[:], ident[:96, :96])
                    as2 = ab.tile([96, 96], F32)
                    nc.scalar.copy(as2[:], as2_ps[:])
                    # mask off-head-block entries to -60
                    t60 = ab.tile([96, 96], F32)
                    nc.vector.tensor_scalar_add(t60[:], as2[:], 60.0)
                    amf = ab.tile([96, 96], F32)
                    nc.vector.tensor_tensor(out=amf[:], in0=t60[:], in1=ws["vmask"][:],
                                            op=Alu.mult)
                    nc.vector.tensor_scalar_add(amf[:], amf[:], -60.0)
                    # rank+1 over full row via pairwise is_ge
                    rnk3 = ab.tile([96, 96 * 96], F32)
                    a_i = amf[:].unsqueeze(1).broadcast_to([96, 96, 96])
                    a_d = amf[:].unsqueeze(2).broadcast_to([96, 96, 96])
                    rvw = rnk3[:].rearrange("p (i d) -> p i d", d=96)
                    nc.vector.tensor_tensor(out=rvw, in0=a_i, in1=a_d, op=Alu.is_ge)
                    rank1 = ab.tile([96, 96], F32)
                    nc.vector.reduce_sum(rank1[:].unsqueeze(2), rvw, axis=AX)
                    sel = ab.tile([96, 96], F32)
                    nc.vector.tensor_tensor(out=sel[:], in0=rank1[:],
                                            in1=dynk[:].broadcast_to([96, 96]), op=Alu.is_le)
                    am = ab.tile([96, 96], F32)
                    t60b = ab.tile([96, 96], F32)
                    nc.vector.tensor_scalar_add(t60b[:], amf[:], 60.0)
                    nc.vector.tensor_tensor(out=am[:], in0=t60b[:], in1=sel[:], op=Alu.mult)
                    nc.vector.tensor_scalar_add(am[:], am[:], -60.0)
                    mx = ab.tile([96, 1], F32)
                    nc.vector.reduce_max(mx[:], am[:], axis=AX)
                    nmx = ab.tile([96, 1], F32)
                    nc.vector.tensor_scalar_mul(nmx[:], mx[:], -1.0)
                    ex = ab.tile([96, 96], F32)
                    nc.scalar.activation(ex[:], am[:], Act.Exp, bias=nmx[:])
                    sme = ab.tile([96, 1], F32)
                    nc.vector.reduce_sum(sme[:], ex[:], axis=AX)
                    rsm = ab.tile([96, 1], F32)
                    nc.vector.reciprocal_approx_fast(rsm[:], sme[:])
                    probs = ab.tile([96, 96], F32)
                    nc.vector.tensor_scalar_mul(probs[:], ex[:], rsm[:])
                    pt_ps = atps.tile([96, 96], F32, tag="atp2")
                    nc.tensor.transpose(pt_ps[:], probs[:], ident[:96, :96])
                    nc.scalar.copy(probsT[:], pt_ps[:])

            # ================= PHASE B5 =================
            with (
                tc.tile_pool(name="b5_rot", bufs=3) as b5r,
                tc.tile_pool(name="b5_ps", bufs=1, space="PSUM") as b5ps,
            ):
                for ci in range(NSC):
                    o0 = ci * 512
                    NN = min(512, S - o0)
                    vch = b5r.tile([96, 512], F32R, tag="vch")
                    nc.sync.dma_start(out=vch[:, :NN], in_=v_sp[:, o0:o0 + NN])
                    av_ps = b5ps.tile([96, 512], F32, tag="avps", bufs=2)
                    nc.tensor.matmul(av_ps[:, :NN], probsT[:], vch[:, :NN],
                                     start=True, stop=True)
                    avs = b5r.tile([96, 512], F32R, tag="avs")
                    nc.scalar.activation(avs[:, :NN], av_ps[:, :NN], Act.Copy,
                                         scale=attn_scale)
                    x2ch = b5r.tile([96, 512], F32R, tag="x2ch")
                    nc.sync.dma_start(out=x2ch[:, :NN], in_=yn2_sp[:, o0:o0 + NN])
                    xpch = [b5r.tile([96, 512], F32R, tag=f"xp{cg}", name=f"xp{cg}") for cg in range(2)]
                    xsq = [b5r.tile([96, 512], F32R, tag=f"xs{cg}", name=f"xs{cg}") for cg in range(2)]
                    for cg in range(2):
                        xcch = b5r.tile([96, 512], F32R, tag=f"xcc{cg}")
                        nc.sync.dma_start(out=xcch[:, :NN], in_=xc_sp[cg][:, o0:o0 + NN])
                        pj_ps = b5ps.tile([96, 512], F32, tag=f"pjps{cg}", bufs=2)
                        nc.tensor.matmul(pj_ps[:, :NN],
                                         ws["proj1T"][:, cg * 96:(cg + 1) * 96],
                                         avs[:, :NN], start=True, stop=False)
                        nc.tensor.matmul(pj_ps[:, :NN],
                                         ws["proj2T"][:, cg * 96:(cg + 1) * 96],
                                         x2ch[:, :NN], start=False, stop=True)
                        nc.vector.scalar_tensor_tensor(
                            out=xpch[cg][:, :NN], in0=pj_ps[:, :NN],
                            scalar=ws["proj_bias"][:, cg:cg + 1], in1=xcch[:, :NN],
                            op0=Alu.add, op1=Alu.add)
                        nc.sync.dma_start(out=xcp_sp[cg][:, o0:o0 + NN],
                                          in_=xpch[cg][:, :NN])
                        nc.scalar.square(xsq[cg][:, :NN], xpch[cg][:, :NN])
                    # LN2 applied here; spill the normalized activations so the
                    # C-phase loop needs no stats broadcast at all
                    mu_ps = b5ps.tile([128, 512], F32, tag="mu2ps", bufs=1)
                    m2_ps = b5ps.tile([128, 512], F32, tag="m22ps", bufs=1)
                    for cg in range(2):
                        nc.tensor.matmul(mu_ps[:, :NN], ws["ones_st"], xpch[cg][:, :NN],
                                         start=(cg == 0), stop=(cg == 1))
                        nc.tensor.matmul(m2_ps[:, :NN], ws["ones_st"], xsq[cg][:, :NN],
                                         start=(cg == 0), stop=(cg == 1))
                    musq = b5r.tile([128, 512], F32, tag="musq2")
                    nc.scalar.square(musq[:, :NN], mu_ps[:, :NN])
                    var = b5r.tile([128, 512], F32, tag="var2")
                    nc.vector.tensor_tensor(out=var[:, :NN], in0=m2_ps[:, :NN],
                                            in1=musq[:, :NN], op=Alu.subtract)
                    sd = b5r.tile([128, 512], F32, tag="sd2")
                    nc.scalar.activation(sd[:, :NN], var[:, :NN], Act.Sqrt, bias=ws["epsv"])
                    rstd = b5r.tile([128, 512], F32, tag="rstd2")
                    nc.vector.reciprocal_approx_fast(rstd[:, :NN], sd[:, :NN])
                    for cg in range(2):
                        td = b5r.tile([96, 512], F32, tag=f"td{cg}")
                        nc.vector.tensor_tensor(out=td[:, :NN], in0=xpch[cg][:, :NN],
                                                in1=mu_ps[:96, :NN], op=Alu.subtract)
                        ynn = b5r.tile([96, 512], F32R, tag=f"ynn{cg}")
                        nc.vector.tensor_tensor(out=ynn[:, :NN], in0=td[:, :NN],
                                                in1=rstd[:96, :NN], op=Alu.mult)
                        nc.sync.dma_start(out=ynn_sp[cg][:, o0:o0 + NN],
                                          in_=ynn[:, :NN])

            _wpab_cm.__exit__(None, None, None)
            # ================= PHASE C =================
            _wpc_cm = tc.tile_pool(name="wpC", bufs=1)
            wpc = _wpc_cm.__enter__()
            _load_w(wpc, C_ONLY_W)
            with tc.tile_pool(name="c_v0", bufs=1) as cv0:
                v0t1 = cv0.tile([128, P3], F32R)
                v0t2 = cv0.tile([128, P3], F32R)
                with (
                    tc.tile_pool(name="c1_rot", bufs=2) as c1r,
                    tc.tile_pool(name="c2_rot", bufs=2) as c2r,
                    tc.tile_pool(name="c_ps", bufs=1, space="PSUM") as cps,
                ):
                    # pad cells must hold -t1/s1 so the bn-folded depthwise
                    # reads zeros in v0_bn space at image borders; only the
                    # border strips are ever read as pad (interior is written
                    # by the fc1 stage below), so skip the full-buffer memset
                    for v0t, pv in ((v0t1, ws["padv1"]), (v0t2, ws["padv2"])):
                        strips = [
                            v0t[:, 0:SLOP + 3 * Wp3],
                            v0t[:, SLOP + (H + 3) * Wp3:P3],
                        ]
                        mid = v0t[:, SLOP + 3 * Wp3:SLOP + (H + 3) * Wp3] \
                            .rearrange("p (r w) -> p r w", w=Wp3)
                        strips.append(mid[:, :, 0:4])
                        strips.append(mid[:, :, 131:134])
                        for st in strips:
                            nc.vector.memset(st.bitcast(F32), 0.0)
                            nc.vector.tensor_scalar_add(st, st, pv)

                    def c1_body(ci):
                        c0 = ci * RC
                        nr_c = min(RC, H - c0)
                        NN = nr_c * W
                        o0 = c0 * W
                        yn2t = [c1r.tile([96, RC * W], F32R, tag=f"cy{cg}", name=f"cy{cg}") for cg in range(2)]
                        for cg in range(2):
                            nc.sync.dma_start(out=yn2t[cg][:, :NN],
                                              in_=ynn_sp[cg][:, o0:o0 + NN])
                        sb0c = pd3(3 + c0)
                        # g0: channels 0:64 -> vg0/ug0
                        fg0 = cps.tile([64, RC * W], F32, tag="fg0", bufs=1)
                        for cg in range(2):
                            nc.tensor.matmul(fg0[:, :NN],
                                             ws["fc1T"][:, cg * 64:(cg + 1) * 64],
                                             yn2t[cg][:, :NN],
                                             start=(cg == 0), stop=(cg == 1))
                        fg2 = cps.tile([64, RC * W], F32, tag="fg2", bufs=1)
                        for cg in range(2):
                            nc.tensor.matmul(fg2[:, :NN],
                                             ws["fc1T"][:, 384 + cg * 64:448 + cg * 64],
                                             yn2t[cg][:, :NN],
                                             start=(cg == 0), stop=(cg == 1))
                        vg0 = c1r.tile([64, RC * W], F32R, tag="vg0")
                        nc.scalar.activation(vg0[:, :NN], fg0[:, :NN], Act.Gelu,
                                             bias=ws["fc1_bias"][0:64, 0:1])
                        ug0 = c1r.tile([64, RC * W], F32R, tag="ug0")
                        nc.scalar.activation(ug0[:, :NN], vg0[:, :NN], Act.Gelu,
                                             bias=ws["g0_bias"], scale=ws["g0_scale"])
                        for r in range(nr_c):
                            d0 = pd3(3 + c0 + r) + 3
                            nc.sync.dma_start(out=vg0_sp[:, d0:d0 + W],
                                              in_=vg0[:, r * W:(r + 1) * W])
                            nc.sync.dma_start(out=ug0_sp[:, d0:d0 + W],
                                              in_=ug0[:, r * W:(r + 1) * W])
                        # g1: channels 64:192 -> v0t1, one strided gelu
                        fg1 = cps.tile([128, RC * W], F32, tag="fg1", bufs=1)
                        for cg in range(2):
                            nc.tensor.matmul(fg1[:, :NN],
                                             ws["fc1T"][:, 128 + cg * 128:256 + cg * 128],
                                             yn2t[cg][:, :NN],
                                             start=(cg == 0), stop=(cg == 1))
                        dv1 = v0t1[:, sb0c:sb0c + nr_c * Wp3].rearrange(
                            "p (r w) -> p r w", w=Wp3)[:, :, 3:3 + W]
                        nc.scalar.activation(
                            dv1, fg1[:, :NN].rearrange("p (r w) -> p r w", w=W),
                            Act.Gelu, bias=ws["fc1_bias"][:, 1:2])
                        # g2: channels 192:256 -> v0t2 halves (second shifted +1)
                        dv2a = v0t2[0:64, sb0c:sb0c + nr_c * Wp3].rearrange(
                            "p (r w) -> p r w", w=Wp3)[:, :, 3:3 + W]
                        nc.scalar.activation(
                            dv2a, fg2[:, :NN].rearrange("p (r w) -> p r w", w=W),
                            Act.Gelu, bias=ws["fc1_bias"][64:128, 0:1])
                        dv2b = v0t2[64:128, sb0c:sb0c + nr_c * Wp3].rearrange(
                            "p (r w) -> p r w", w=Wp3)[:, :, 4:4 + W]
                        nc.scalar.activation(
                            dv2b, fg2[:, :NN].rearrange("p (r w) -> p r w", w=W),
                            Act.Gelu, bias=ws["fc1_bias"][64:128, 0:1])

                    def c2_body(ci):
                        c0 = ci * RC
                        nr_c = min(RC, H - c0)
                        N = nr_c * Wp3
                        NN = nr_c * W
                        sb0 = pd3(3 + c0)
                        ps_a = cps.tile([128, RC * Wp3], F32, tag="psa", bufs=2)
                        for t in range(25):
                            dy, dx = t // 5 - 2, t % 5 - 2
                            o = sb0 + dy * Wp3 + dx
                            nc.tensor.matmul(ps_a[:, :N],
                                             ws["pair_diag"][:, t * 128:(t + 1) * 128],
                                             v0t1[:, o:o + N],
                                             start=(t == 0), stop=(t == 24))
                        ps_b = cps.tile([64, RC * Wp3], F32, tag="psb", bufs=1)
                        for i, (dy, dxa, hasb) in enumerate(dw3_passes):
                            o = sb0 + dy * Wp3 + dxa
                            nc.tensor.matmul(ps_b[:, :N],
                                             ws["dw3_diag"][:, i * 64:(i + 1) * 64],
                                             v0t2[:, o:o + N],
                                             start=(i == 0), stop=(i == len(dw3_passes) - 1))

                        def inner(ap_flat, lo, hi):
                            # interior view of a PSUM chunk (starts at free 0)
                            return ap_flat[lo:hi, :N].rearrange(
                                "p (r w) -> p r w", w=Wp3)[:, :, 3:3 + W]

                        def inner_v0(ap_flat, lo, hi):
                            # interior view of the padded v0 buffers at this chunk
                            return ap_flat[lo:hi, sb0:sb0 + N].rearrange(
                                "p (r w) -> p r w", w=Wp3)[:, :, 3:3 + W]

                        ug_a = c2r.tile([128, RC * W], F32R, tag="uga")
                        ug_b = c2r.tile([128, RC * W], F32R, tag="ugb")
                        vb_a = c2r.tile([128, RC * W], F32R, tag="vba")
                        vb_b = c2r.tile([128, RC * W], F32R, tag="vbb")
                        g0v = c2r.tile([64, RC * W], F32R, tag="g0v")
                        src3 = vg0_sp[:, sb0:sb0 + N].rearrange(
                            "p (r w) -> p r w", w=Wp3)[:, :, 3:3 + W]
                        nc.sync.dma_start(
                            out=g0v[:, :NN].rearrange("p (r w) -> p r w", w=W), in_=src3)
                        usrc3 = ug0_sp[:, sb0:sb0 + N].rearrange(
                            "p (r w) -> p r w", w=Wp3)[:, :, 3:3 + W]
                        nc.sync.dma_start(
                            out=ug_a[0:64, :NN].rearrange("p (r w) -> p r w", w=W), in_=usrc3)
                        nc.scalar.activation(
                            ug_a[64:128, :NN].rearrange("p (r w) -> p r w", w=W),
                            inner(ps_a, 0, 64), Act.Gelu, bias=ws["pair_bias"][0:64])
                        nc.scalar.activation(
                            ug_b[0:64, :NN].rearrange("p (r w) -> p r w", w=W),
                            inner(ps_a, 64, 128), Act.Gelu, bias=ws["pair_bias"][64:128])
                        nc.scalar.activation(
                            ug_b[64:128, :NN].rearrange("p (r w) -> p r w", w=W),
                            inner(ps_b, 0, 64), Act.Gelu, bias=ws["dw3_bias"])
                        nc.vector.tensor_scalar(out=vb_a[0:64, :NN], in0=g0v[:, :NN],
                                                scalar1=ws["s1a"][0:64],
                                                scalar2=ws["t1a"][0:64],
                                                op0=Alu.mult, op1=Alu.add)
                        nc.vector.tensor_scalar(out=vb_a[64:128, :NN],
                                                in0=inner_v0(v0t1, 0, 64),
                                                scalar1=ws["s1a"][64:128],
                                                scalar2=ws["t1a"][64:128],
                                                op0=Alu.mult, op1=Alu.add)
                        nc.vector.tensor_scalar(out=vb_b[0:64, :NN],
                                                in0=inner_v0(v0t1, 64, 128),
                                                scalar1=ws["s1b"][0:64],
                                                scalar2=ws["t1b"][0:64],
                                                op0=Alu.mult, op1=Alu.add)
                        nc.vector.tensor_scalar(out=vb_b[64:128, :NN],
                                                in0=inner_v0(v0t2, 0, 64),
                                                scalar1=ws["s1b"][64:128],
                                                scalar2=ws["t1b"][64:128],
                                                op0=Alu.mult, op1=Alu.add)
                        # z1 = ug * vb computed in place into the vb tiles
                        nc.vector.tensor_tensor(out=vb_a[:, :NN], in0=ug_a[:, :NN],
                                                in1=vb_a[:, :NN], op=Alu.mult)
                        nc.vector.tensor_tensor(out=vb_b[:, :NN], in0=ug_b[:, :NN],
                                                in1=vb_b[:, :NN], op=Alu.mult)
                        occ = {}
                        tpps = {}
                        for cg in range(2):
                            # ops [96, 0:384] and the output-transpose psum
                            # [128, 384:480] share one bank
                            cmb = cps.tile([128, RC * W + 96], F32, tag=f"cmb{cg}",
                                           bufs=1)
                            ops = cmb[0:96, 0:RC * W]
                            ops = cmb[0:96, 0:NN]
                            tpps[cg] = cmb[:, RC * W:RC * W + 96]
                            nc.tensor.matmul(ops,
                                             ws["fc2aT"][:, (cg * 2) * 96:(cg * 2 + 1) * 96],
                                             vb_a[:, :NN], start=True, stop=False)
                            nc.tensor.matmul(ops,
                                             ws["fc2aT"][:, (cg * 2 + 1) * 96:(cg * 2 + 2) * 96],
                                             vb_b[:, :NN], start=False, stop=False)
                            nc.tensor.matmul(ops,
                                             ws["fc2bT_g0"][:, cg * 96:(cg + 1) * 96],
                                             g0v[:, :NN], start=False, stop=False)
                            opsv = ops.rearrange("p (r w) -> p r w", w=W)
                            nc.tensor.matmul(opsv,
                                             ws["fc2bT_g12"][:, cg * 96:(cg + 1) * 96],
                                             inner_v0(v0t1, 0, 128), start=False, stop=False)
                            nc.tensor.matmul(opsv,
                                             ws["fc2bT_g3"][:, cg * 96:(cg + 1) * 96],
                                             inner_v0(v0t2, 0, 64), start=False, stop=True)
                            xrch = c2r.tile([96, RC * W], F32R, tag=f"xr{cg}", bufs=1)
                            nc.sync.dma_start(out=xrch[:, :NN],
                                              in_=xcp_sp[cg][:, c0 * W:c0 * W + NN])
                            ob = c2r.tile([96, RC * W], F32, tag=f"ob{cg}", bufs=1)
                            nc.vector.tensor_scalar(out=ob[:, :NN], in0=ops,
                                                    scalar1=ws["s3v"][:, cg:cg + 1],
                                                    scalar2=ws["out_bias"][:, cg:cg + 1],
                                                    op0=Alu.mult, op1=Alu.add)
                            nc.vector.tensor_tensor(out=ob[:, :NN], in0=ob[:, :NN],
                                                    in1=xrch[:, :NN], op=Alu.add)
                            occ[cg] = ob
                        # transpose to pixel-major [W, EMBED] per image row so the
                        # output DMA writes contiguous 768B lines instead of a
                        # 4B-per-element scatter
                        outT = c2r.tile([W, RC * EMBED], F32, tag="outT", bufs=1)
                        for r in range(nr_c):
                            for cg in range(2):
                                nc.tensor.transpose(
                                    tpps[cg], occ[cg][:, r * W:(r + 1) * W],
                                    ws["ident"][:96, :96])
                                nc.scalar.copy(
                                    outT[:, r * EMBED + cg * 96:r * EMBED + (cg + 1) * 96],
                                    tpps[cg])
                        for r in range(nr_c):
                            nc.sync.dma_start(
                                out=out_t[(c0 + r) * W:(c0 + r + 1) * W, :],
                                in_=outT[:, r * EMBED:(r + 1) * EMBED])

                    # interleave: fc1/gelu of chunk it overlaps the
                    # TensorE-bound depthwise/fc2 of chunk it-2
                    for it in range(NCH + 2):
                        if it < NCH:
                            c1_body(it)
                        if it >= 2:
                            c2_body(it - 2)
            _wpc_cm.__exit__(None, None, None)
    return out_t.name


# ----------------------------------------------------------------------------
# host entry
# ----------------------------------------------------------------------------

_CACHE = {}


def make_program(H, W, n_cores, attn_scale, dw3_passes):
    key = (H, W, n_cores, round(attn_scale, 9))
    if key in _CACHE:
        return _CACHE[key]
    nc = bacc.Bacc("TRN2", target_bir_lowering=False, debug=False, num_devices=n_cores)
    out_name = build(nc, H, W, n_cores, attn_scale, dw3_passes)
    nc.compile()
    _CACHE[key] = (nc, out_name)
    return nc, out_name


def make_in_maps(inputs):
    x = np.asarray(inputs["x"], np.float32)
    B = x.shape[0]
    wdict = _prep_weights({k: np.asarray(v) for k, v in inputs.items()})
    base = {}
    for k, (shp, d) in WSPEC.items():
        base["w_" + k] = wdict[k][0].reshape(shp)
    in_maps = []
    for b in range(B):
        m = dict(base)
        m["x"] = np.ascontiguousarray(x[b])
        in_maps.append(m)
    return in_maps, wdict


def kernel(**inputs):
    x = np.asarray(inputs["x"], np.float32)
    B, H, W, C = x.shape
    in_maps, wdict = make_in_maps(inputs)
    nc, out_name = make_program(H, W, B, wdict["_attn_scale"][0],
                                wdict["_dw3_passes"][0])
    res = bass_utils.run_bass_kernel_spmd(nc, in_maps, core_ids=list(range(B)))
    return np.stack([res.results[b][out_name].reshape(H, W, C) for b in range(B)])



# revision 57
# speedup vs baseline: 1.1528x; 1.1528x over previous
"""Trainium2 Bass kernel for nn_Block_87351044866235 (sparse_attention).

Data-parallel over batch: 8 samples -> 8 NeuronCores. Channel-major
layout [C, H*W] on chip; depthwise convs as diagonal fp32r matmuls on
TensorE; 1x1 convs as fp32r matmuls; LN stats via ones-matmuls; q/k gram
via hi/lo bf16 split + DMA-xbar transposes; dynamic-k gate mean via a
scalar AllReduce.
"""
import sys, os

for _p in ("/opt/trn_rl_repo", "/root/.axon_site/_ro/trn_rl_repo"):
    if os.path.isdir(_p) and _p not in sys.path:
        sys.path.append(_p)

import numpy as np
import concourse.bass as bass
import concourse.bacc as bacc
import concourse.tile as tile
from concourse import mybir
from concourse import bass_utils

try:
    from concourse import tile_utils as _tu
    _tu.max_sbuf_usage = 208 * 1024
except Exception:
    pass

dt = mybir.dt
Alu = mybir.AluOpType
Act = mybir.ActivationFunctionType
AX = mybir.AxisListType.X

EMBED, PDIM, HEADS, HID = 192, 96, 8, 256
CPH = PDIM // HEADS  # 12
SLOP = 8
RC = 3    # conv output rows per chunk
BR = 12   # rows per band


F32, F32R, BF16 = dt.float32, dt.float32r, dt.bfloat16


def _ceil(a, b):
    return (a + b - 1) // b


# ----------------------------------------------------------------------------
# host-side weight prep: everything 2D [partitions, free]
# ----------------------------------------------------------------------------

def _prep_weights(p):
    w = {}
    f32r = lambda a: (np.ascontiguousarray(a, np.float32), F32R)
    f32 = lambda a: (np.ascontiguousarray(a, np.float32), F32)
    eps_bn = 1e-5

    w["ident"] = f32(np.eye(128, dtype=np.float32))
    w["identr"] = f32r(np.eye(128, dtype=np.float32))

    # pos depthwise diag: [96, (t*2+cg)*96]
    pw = p["pos_w"][:, 0]  # [192,3,3]
    pos_d = np.zeros((96, 18 * 96), np.float32)
    for t in range(9):
        dy, dx = t // 3 - 1, t % 3 - 1
        for cg in range(2):
            pos_d[:, (t * 2 + cg) * 96:(t * 2 + cg + 1) * 96] = \
                np.diag(pw[cg * 96:(cg + 1) * 96, dy + 1, dx + 1])
    w["pos_diag"] = f32r(pos_d)
    w["pos_b"] = f32(p["pos_b"].reshape(2, 96).T)  # [96, 2]

    g1v, b1v = p["ln1_g"], p["ln1_b"]
    qw = p["qkv_w"][:, :, 0, 0]  # [288, 96]
    qw_eff = qw * g1v[None, :96]
    w["qkv_wT"] = f32r(np.concatenate(
        [qw_eff[j * 96:(j + 1) * 96].T for j in range(3)], axis=1))  # [96, 3*96]
    w["qkv_bias"] = f32((qw @ b1v[:96]).reshape(3, 96).T)  # [96, 3]

    qdw = p["qkv_dw_w"][:, 0]  # [288,3,3]
    qdw_d = np.zeros((96, 27 * 96), np.float32)
    for t in range(9):
        dy, dx = t // 3 - 1, t % 3 - 1
        for j in range(3):
            qdw_d[:, (t * 3 + j) * 96:(t * 3 + j + 1) * 96] = \
                np.diag(qdw[j * 96:(j + 1) * 96, dy + 1, dx + 1])
    w["qdw_diag"] = f32r(qdw_d)

    gw1 = p["gate_w1"][:, :, 0, 0]  # [96, 192]
    gw1_eff = gw1 * g1v[None, :]
    w["gate_w1T"] = f32r(np.concatenate(
        [gw1_eff[:, cg * 96:(cg + 1) * 96].T for cg in range(2)], axis=1))  # [96, 192]
    w["gate_b1"] = f32((p["gate_b1"] + gw1 @ b1v).reshape(96, 1))
    w["gate_w2T"] = f32r(p["gate_w2"][:, :, 0, 0].T.copy())  # [96,1]
    w["gate_b2"] = f32(p["gate_b2"].reshape(1, 1))

    pj = p["proj_w"][:, :, 0, 0]
    pj1, pj2 = pj[:, :96], pj[:, 96:] * g1v[None, 96:]
    w["proj1T"] = f32r(np.concatenate(
        [pj1[cg * 96:(cg + 1) * 96].T for cg in range(2)], axis=1))  # [96, 192]
    w["proj2T"] = f32r(np.concatenate(
        [pj2[cg * 96:(cg + 1) * 96].T for cg in range(2)], axis=1))
    w["proj_bias"] = f32((pj[:, 96:] @ b1v[96:]).reshape(2, 96).T)  # [96, 2]

    attn_scale = float(p["attn1"][0] + p["attn2"][0] + p["attn3"][0] + p["attn4"][0])
    w["_attn_scale"] = (attn_scale, None)
    w["tempvec"] = f32(np.repeat(p["temperature"].reshape(HEADS), CPH).reshape(96, 1))

    g2v, b2v = p["ln2_g"], p["ln2_b"]
    f1 = p["fc1_w"][:, :, 0, 0]  # [256, 192]
    f1_eff = f1 * g2v[None, :]
    # channel groups 64|128|64 so each group's gelu lands in one strided op:
    # g0 -> vg0 (ch 0:64), g1 -> v0t1 (ch 64:192), g2 -> v0t2 (ch 192:256)
    fc1 = np.zeros((96, 512), np.float32)
    fc1[:, 0:64] = f1_eff[0:64, 0:96].T
    fc1[:, 64:128] = f1_eff[0:64, 96:192].T
    fc1[:, 128:256] = f1_eff[64:192, 0:96].T
    fc1[:, 256:384] = f1_eff[64:192, 96:192].T
    fc1[:, 384:448] = f1_eff[192:256, 0:96].T
    fc1[:, 448:512] = f1_eff[192:256, 96:192].T
    w["fc1T"] = f32r(fc1)
    fb = f1 @ b2v
    fbias = np.zeros((128, 2), np.float32)
    fbias[0:64, 0] = fb[0:64]
    fbias[64:128, 0] = fb[192:256]
    fbias[:, 1] = fb[64:192]
    w["fc1_bias"] = f32(fbias)

    s1 = p["bn1_g"] / np.sqrt(p["bn1_v"] + eps_bn)
    t1 = p["bn1_b"] - p["bn1_m"] * s1
    s2 = p["bn2_g"] / np.sqrt(p["bn2_v"] + eps_bn)
    t2 = p["bn2_b"] - p["bn2_m"] * s2
    s3 = p["bn3_g"] / np.sqrt(p["bn3_v"] + eps_bn)
    t3 = p["bn3_b"] - p["bn3_m"] * s3

    dw1w, dw2w, dw3w = p["dw1_w"][:, 0], p["dw2_w"][:, 0], p["dw3_w"][:, 0]
    dw1b, dw2b, dw3b = p["dw1_b"], p["dw2_b"], p["dw3_b"]
    s1g = [s1[i * 64:(i + 1) * 64] for i in range(4)]
    t1g = [t1[i * 64:(i + 1) * 64] for i in range(4)]

    def pair_tap_diag(t):
        dy, dx = t // 5 - 2, t % 5 - 2
        v = np.zeros(128, np.float32)
        d2 = dw2w[:, dy + 2, dx + 2] * s1g[2]
        if dy == 0 and dx == 0:
            d2 = d2 + s1g[2]
        v[64:] = d2
        if -1 <= dy <= 1 and -1 <= dx <= 1:
            d1 = dw1w[:, dy + 1, dx + 1] * s1g[1]
            if dy == 0 and dx == 0:
                d1 = d1 + s1g[1]
            v[:64] = d1
        return v

    pair_d = np.zeros((128, 25 * 128), np.float32)
    for t in range(25):
        pair_d[:, t * 128:(t + 1) * 128] = np.diag(pair_tap_diag(t))
    w["pair_diag"] = f32r(pair_d)
    bc1 = t1g[1] * dw1w.sum((1, 2)) + dw1b + t1g[1]
    bc2 = t1g[2] * dw2w.sum((1, 2)) + dw2b + t1g[2]
    w["pair_bias"] = f32(np.concatenate([bc1, bc2]).reshape(128, 1))

    # rows 64:128 of v0t2 hold the same data stored shifted +1, so a read at
    # AP offset (dy, dxa) yields tap (dy, dxa-1) for those rows.
    dw3_passes = []
    for dy in range(-3, 4):
        for dxa in (-2, 0, 2):
            dw3_passes.append((dy, dxa, True))
        dw3_passes.append((dy, 3, False))
    dw3_d = np.zeros((128, len(dw3_passes) * 64), np.float32)
    for i, (dy, dxa, hasb) in enumerate(dw3_passes):
        wa = dw3w[:, dy + 3, dxa + 3] * s1g[3]
        if dy == 0 and dxa == 0:
            wa = wa + s1g[3]
        dw3_d[:64, i * 64:(i + 1) * 64] = np.diag(wa)
        if hasb:
            wb = dw3w[:, dy + 3, dxa - 1 + 3] * s1g[3]
            if dy == 0 and dxa - 1 == 0:
                wb = wb + s1g[3]
            dw3_d[64:, i * 64:(i + 1) * 64] = np.diag(wb)
    w["dw3_diag"] = f32r(dw3_d)
    w["_dw3_passes"] = (dw3_passes, None)
    w["dw3_bias"] = f32((t1g[3] * dw3w.sum((1, 2)) + dw3b + t1g[3]).reshape(64, 1))

    d0w, d0b = p["dw0_w"][:, 0, 0, 0], p["dw0_b"]
    w["g0_scale"] = f32(((d0w + 1.0) * s1g[0]).reshape(64, 1))
    w["g0_bias"] = f32(((d0w + 1.0) * t1g[0] + d0b).reshape(64, 1))

    f2 = p["fc2_w"][:, :, 0, 0]  # [192, 256]
    f2a = f2 * s2[None, :]
    f2b = f2 * (t2 * s1)[None, :]
    cstv = f2 @ (t2 * t1)
    fc2a = np.zeros((128, 4 * 96), np.float32)
    for cg in range(2):
        for kg in range(2):
            fc2a[:, (cg * 2 + kg) * 96:(cg * 2 + kg + 1) * 96] = \
                f2a[cg * 96:(cg + 1) * 96, kg * 128:(kg + 1) * 128].T
    w["fc2aT"] = f32r(fc2a)
    w["fc2bT_g0"] = f32r(np.concatenate(
        [f2b[cg * 96:(cg + 1) * 96, 0:64].T for cg in range(2)], axis=1))    # [64, 192]
    w["fc2bT_g12"] = f32r(np.concatenate(
        [f2b[cg * 96:(cg + 1) * 96, 64:192].T for cg in range(2)], axis=1))  # [128, 192]
    w["fc2bT_g3"] = f32r(np.concatenate(
        [f2b[cg * 96:(cg + 1) * 96, 192:256].T for cg in range(2)], axis=1))  # [64, 192]
    w["s3v"] = f32(np.stack([s3[:96], s3[96:]], axis=1))          # [96, 2]
    w["out_bias"] = f32(np.stack([(s3 * 0 + t3 + s3 * cstv * 0)[:96], (t3)[96:]], axis=1))
    # careful: out = s3*(psum + cst) + t3 + xc' ; psum excludes cst, so bias = s3*cst + t3
    ob = s3 * cstv + t3
    w["out_bias"] = f32(np.stack([ob[:96], ob[96:]], axis=1))     # [96, 2]

    sg = np.where(s1 == 0, 1.0, s1)
    padv = -t1 / sg
    w["padv1"] = f32(np.concatenate([padv[64:128], padv[128:192]]).reshape(128, 1))
    w["padv2"] = f32(np.concatenate([padv[192:256], padv[192:256]]).reshape(128, 1))
    w["s1a"] = f32(s1[:128].reshape(128, 1))
    w["s1b"] = f32(s1[128:].reshape(128, 1))
    w["t1a"] = f32(t1[:128].reshape(128, 1))
    w["t1b"] = f32(t1[128:].reshape(128, 1))

    w["ones_st"] = f32r(np.full((96, 128), 1.0 / EMBED, np.float32))
    w["epsv"] = f32(np.full((128, 1), 1e-6, np.float32))
    vm = np.zeros((96, 96), np.float32)
    for h in range(HEADS):
        vm[h * CPH:(h + 1) * CPH, h * CPH:(h + 1) * CPH] = 1.0
    w["vmask"] = f32(vm)
    return w


WSPEC = {
    "ident": ([128, 128], F32), "identr": ([128, 128], F32R),
    "pos_diag": ([96, 18 * 96], F32R),
    "pos_b": ([96, 2], F32), "qkv_wT": ([96, 3 * 96], F32R),
    "qkv_bias": ([96, 3], F32), "qdw_diag": ([96, 27 * 96], F32R),
    "gate_w1T": ([96, 192], F32R), "gate_b1": ([96, 1], F32),
    "gate_w2T": ([96, 1], F32R), "gate_b2": ([1, 1], F32),
    "proj1T": ([96, 192], F32R), "proj2T": ([96, 192], F32R),
    "proj_bias": ([96, 2], F32), "tempvec": ([96, 1], F32),
    "fc1T": ([96, 4 * 128], F32R), "fc1_bias": ([128, 2], F32),
    "pair_diag": ([128, 25 * 128], F32R), "pair_bias": ([128, 1], F32),
    "dw3_diag": ([128, 28 * 64], F32R), "dw3_bias": ([64, 1], F32),
    "g0_scale": ([64, 1], F32), "g0_bias": ([64, 1], F32),
    "fc2aT": ([128, 4 * 96], F32R), "fc2bT_g0": ([64, 192], F32R),
    "fc2bT_g12": ([128, 192], F32R), "fc2bT_g3": ([64, 192], F32R),
    "s3v": ([96, 2], F32), "out_bias": ([96, 2], F32),
    "padv1": ([128, 1], F32),
    "padv2": ([128, 1], F32),
    "s1a": ([128, 1], F32), "s1b": ([128, 1], F32),
    "t1a": ([128, 1], F32), "t1b": ([128, 1], F32),
    "ones_st": ([96, 128], F32R),
    "epsv": ([128, 1], F32),
    "vmask": ([96, 96], F32),
}


# ----------------------------------------------------------------------------
# device kernel
# ----------------------------------------------------------------------------

def build(nc, H, W, n_cores, attn_scale, dw3_passes):
    S = H * W
    Wp1 = W + 2
    P1B = (BR + 2) * Wp1 + 2 * SLOP   # band buffer (pad1)
    Wp3, Hp3 = W + 6, H + 6
    P3 = Hp3 * Wp3 + 2 * SLOP
    NCH = _ceil(H, RC)
    NB = _ceil(H, BR)
    NSC = _ceil(S, 512)
    GCH = 512 // W                    # gate chunk rows (512 cols)
    NGC_PER_BAND = _ceil(BR, GCH)

    x_t = nc.dram_tensor("x", [H, W, EMBED], F32, kind="ExternalInput")
    out_t = nc.dram_tensor("out", [S, EMBED], F32, kind="ExternalOutput")
    wt = {k: nc.dram_tensor("w_" + k, shp, d, kind="ExternalInput")
          for k, (shp, d) in WSPEC.items()}

    def pd3(r):
        return SLOP + r * Wp3

    with tile.TileContext(nc) as tc:
        C_ONLY_W = ['fc1T', 'fc1_bias', 'pair_diag', 'pair_bias', 'dw3_diag', 'dw3_bias', 'g0_scale', 'g0_bias', 'fc2aT', 'fc2bT_g0', 'fc2bT_g12', 'fc2bT_g3', 's3v', 'out_bias', 's1a', 's1b', 't1a', 't1b', 'padv1', 'padv2']
        with (
            tc.tile_pool(name="dram", bufs=1, space="DRAM") as dram,
            tc.tile_pool(name="persist", bufs=1) as pers,
        ):
            ws = {}

            def _load_w(pool, names):
                for k in names:
                    shp, d = WSPEC[k]
                    tl = pool.tile(shp, d, tag="w_" + k, name="w_" + k)
                    nc.sync.dma_start(out=tl[:], in_=wt[k][:])
                    ws[k] = tl


            yn1_sp = dram.tile([96, S], F32R)
            yn2_sp = dram.tile([96, S], F32R)
            xc_sp = [dram.tile([96, S], F32R, name=f"xc_sp{i}") for i in range(2)]
            v_sp = dram.tile([96, S], F32R)
            xcp_sp = [dram.tile([96, S], F32R, name=f"xcp_sp{i}") for i in range(2)]
            ynn_sp = [dram.tile([96, S], F32R, name=f"ynn_sp{i}") for i in range(2)]
            vg0_sp = dram.tile([64, P3], F32R)
            ug0_sp = dram.tile([64, P3], F32R)
            dbg_sp = {nm: dram.tile([128, S], F32, name="dbg_" + nm)
                      for nm in ("uga", "ugb", "vba", "vbb", "z1a", "z1b")} \
                if getattr(build, "DEBUG", False) else None
            cc_in = dram.tile([1, 1], F32)
            cc_out = dram.tile([1, 1], F32)

            gsum = pers.tile([1, NB * NGC_PER_BAND + 8], F32)
            nc.vector.memset(gsum[:], 0.0)
            dynk = pers.tile([96, 1], F32)
            probsT = pers.tile([96, 96], F32R)
            _load_w(pers, ["ident", "identr"])
            ident = ws["ident"]
            identr = ws["identr"]

            # ================= PHASE A =================
            _wpab_cm = tc.tile_pool(name="wpAB", bufs=1)
            wpab = _wpab_cm.__enter__()
            _load_w(wpab, [k for k in WSPEC if k not in C_ONLY_W and k != "ident"])
            with (
                tc.tile_pool(name="pa_band", bufs=2) as pab,
                tc.tile_pool(name="pa_rot", bufs=3) as par,
                tc.tile_pool(name="pa_ps", bufs=2, space="PSUM") as paps,
                tc.tile_pool(name="pa_ps2", bufs=2, space="PSUM") as paps2,
            ):
                for b in range(NB):
                    r0, r1 = b * BR, min((b + 1) * BR, H)
                    xband = [pab.tile([96, P1B], F32R, tag=f"xb{cg}", name=f"xb{cg}") for cg in range(2)]
                    for cg in range(2):
                        nc.vector.memset(xband[cg][:].bitcast(F32), 0.0)
                    for rr in range(max(r0 - 1, 0), min(r1 + 1, H)):
                        xrow = par.tile([W, EMBED], F32, tag="xrow")
                        nc.sync.dma_start(out=xrow[:], in_=x_t[rr])
                        boff = SLOP + (rr - (r0 - 1)) * Wp1 + 1
                        for cg in range(2):
                            tps = paps2.tile([96, W], F32, tag="tps")
                            nc.tensor.transpose(tps[:], xrow[:, cg * 96:(cg + 1) * 96],
                                                ident[:W, :W])
                            nc.scalar.copy(xband[cg][:, boff:boff + W], tps[:])
                    for c0 in range(r0, r1, RC):
                        nr_c = min(RC, H - c0)
                        N = nr_c * Wp1
                        NN = nr_c * W
                        sb0 = SLOP + (c0 - r0 + 1) * Wp1
                        xc_ch = [par.tile([96, RC * W], F32R, tag=f"xc{cg}", name=f"xc{cg}") for cg in range(2)]
                        xsq = [par.tile([96, RC * W], F32R, tag=f"xq{cg}", name=f"xq{cg}") for cg in range(2)]
                        for cg in range(2):
                            ps = paps.tile([96, RC * Wp1], F32, tag="posps")
                            for t in range(9):
                                dy, dx = t // 3 - 1, t % 3 - 1
                                o = sb0 + dy * Wp1 + dx
                                nc.tensor.matmul(
                                    ps[:, :N],
                                    ws["pos_diag"][:, (t * 2 + cg) * 96:(t * 2 + cg + 1) * 96],
                                    xband[cg][:, o:o + N],
                                    start=(t == 0), stop=(t == 8))
                            ps_int = ps[:, :N].rearrange("p (r w) -> p r w", w=Wp1)[:, :, 1:1 + W]
                            xb_int = xband[cg][:, sb0:sb0 + N] \
                                .rearrange("p (r w) -> p r w", w=Wp1)[:, :, 1:1 + W]
                            xcv = xc_ch[cg][:, :NN].rearrange("p (r w) -> p r w", w=W)
                            nc.vector.scalar_tensor_tensor(
                                out=xcv, in0=ps_int, scalar=ws["pos_b"][:, cg:cg + 1],
                                in1=xb_int, op0=Alu.add, op1=Alu.add)
                            nc.scalar.square(xsq[cg][:, :NN], xc_ch[cg][:, :NN])
                        mu_ps = paps.tile([128, RC * W], F32, tag="mups")
                        m2_ps = paps.tile([128, RC * W], F32, tag="m2ps")
                        for cg in range(2):
                            nc.tensor.matmul(mu_ps[:, :NN], ws["ones_st"], xc_ch[cg][:, :NN],
                                             start=(cg == 0), stop=(cg == 1))
                            nc.tensor.matmul(m2_ps[:, :NN], ws["ones_st"], xsq[cg][:, :NN],
                                             start=(cg == 0), stop=(cg == 1))
                        musq = par.tile([128, RC * W], F32, tag="musq")
                        nc.scalar.square(musq[:, :NN], mu_ps[:, :NN])
                        var = par.tile([128, RC * W], F32, tag="var")
                        nc.vector.tensor_tensor(out=var[:, :NN], in0=m2_ps[:, :NN],
                                                in1=musq[:, :NN], op=Alu.subtract)
                        sd = par.tile([128, RC * W], F32, tag="sd")
                        nc.scalar.activation(sd[:, :NN], var[:, :NN], Act.Sqrt, bias=ws["epsv"])
                        rstd = par.tile([128, RC * W], F32, tag="rstd")
                        nc.vector.reciprocal_approx_fast(rstd[:, :NN], sd[:, :NN])
                        for cg in range(2):
                            tdf = par.tile([96, RC * W], F32, tag=f"td{cg}")
                            nc.vector.tensor_tensor(out=tdf[:, :NN], in0=xc_ch[cg][:, :NN],
                                                    in1=mu_ps[:96, :NN], op=Alu.subtract)
                            ynch = par.tile([96, RC * W], F32R, tag=f"yn{cg}")
                            nc.vector.tensor_tensor(out=ynch[:, :NN], in0=tdf[:, :NN],
                                                    in1=rstd[:96, :NN], op=Alu.mult)
                            sp = yn1_sp if cg == 0 else yn2_sp
                            nc.sync.dma_start(out=sp[:, c0 * W:c0 * W + NN],
                                              in_=ynch[:, :NN])
                            nc.sync.dma_start(out=xc_sp[cg][:, c0 * W:c0 * W + NN],
                                              in_=xc_ch[cg][:, :NN])

            # ================= PHASE B =================
            with (
                tc.tile_pool(name="pb_band", bufs=2) as pbb,
                tc.tile_pool(name="pb_rot", bufs=3) as pbr,
                tc.tile_pool(name="gram_ps", bufs=1, space="PSUM") as gpsp,
            ):
                # one bank: [0:96]=q.q, [96:192]=q.k, [192:288]=k.k
                g1_ps = gpsp.tile([96, 288], F32)
                with (
                    tc.tile_pool(name="pb_psg", bufs=1, space="PSUM") as pbpsg,
                    tc.tile_pool(name="pb_ps", bufs=2, space="PSUM") as pbps,
                ):
                    for b in range(NB):
                        r0, r1 = b * BR, min((b + 1) * BR, H)
                        ylo, yhi = max(r0 - 1, 0), min(r1 + 1, H)
                        ynb = [pbb.tile([96, (BR + 2) * W], F32R, tag=f"ynb{cg}", name=f"ynb{cg}")
                               for cg in range(2)]
                        for cg in range(2):
                            sp = yn1_sp if cg == 0 else yn2_sp
                            nc.sync.dma_start(
                                out=ynb[cg][:, (ylo - r0 + 1) * W:(yhi - r0 + 1) * W],
                                in_=sp[:, ylo * W:yhi * W])
                        # gate (512-col chunks over rows [r0, r1))
                        for gi in range(NGC_PER_BAND):
                            gr0 = r0 + gi * GCH
                            if gr0 >= r1:
                                break
                            ngr = min(GCH, r1 - gr0)
                            NG = ngr * W
                            yo = (gr0 - r0 + 1) * W
                            gps = pbpsg.tile([96, 512], F32, tag="gps")
                            for cg in range(2):
                                nc.tensor.matmul(gps[:, :NG],
                                                 ws["gate_w1T"][:, cg * 96:(cg + 1) * 96],
                                                 ynb[cg][:, yo:yo + NG],
                                                 start=(cg == 0), stop=(cg == 1))
                            g1s = pbr.tile([96, 512], F32R, tag="g1s")
                            nc.scalar.activation(g1s[:, :NG], gps[:, :NG], Act.Relu,
                                                 bias=ws["gate_b1"])
                            g2ps = pbpsg.tile([1, 512], F32, tag="g2ps")
                            nc.tensor.matmul(g2ps[:, :NG], ws["gate_w2T"], g1s[:, :NG],
                                             start=True, stop=True)
                            sgt = pbr.tile([1, 512], F32, tag="sgt")
                            idx = b * NGC_PER_BAND + gi
                            nc.scalar.activation(sgt[:, :NG], g2ps[:, :NG], Act.Sigmoid,
                                                 bias=ws["gate_b2"],
                                                 accum_out=gsum[0:1, idx:idx + 1])
                        # qkv0 band
                        qkv0 = [pbb.tile([96, P1B], F32R, tag=f"qk0{j}", name=f"qk0{j}") for j in range(3)]
                        for j in range(3):
                            nc.vector.memset(qkv0[j][:].bitcast(F32), 0.0)
                        for rr in range(ylo, yhi, 2):
                            nrw = min(2, yhi - rr)
                            NQ = nrw * W
                            for j in range(3):
                                qps = pbps.tile([96, 2 * W], F32, tag="qps", bufs=1)
                                nc.tensor.matmul(qps[:, :NQ],
                                                 ws["qkv_wT"][:, j * 96:(j + 1) * 96],
                                                 ynb[0][:, (rr - r0 + 1) * W:(rr - r0 + 1) * W + NQ],
                                                 start=True, stop=True)
                                dst = SLOP + (rr - r0 + 1) * Wp1 + 1
                                dview = qkv0[j][:, dst:dst + nrw * Wp1] \
                                    .rearrange("p (r w) -> p r w", w=Wp1)[:, :, 0:W]
                                nc.scalar.activation(
                                    dview, qps[:, :NQ].rearrange("p (r w) -> p r w", w=W),
                                    Act.Identity, bias=ws["qkv_bias"][:, j:j + 1])
                        # depthwise, then per-row TensorE transpose + fp32 gram
                        for c0 in range(r0, r1, RC):
                            nr_c = min(RC, H - c0)
                            N = nr_c * Wp1
                            NN = nr_c * W
                            sb0 = SLOP + (c0 - r0 + 1) * Wp1
                            qk_ch = {}
                            for j in range(3):
                                ps = pbps.tile([96, RC * Wp1], F32, tag="dwps")
                                for t in range(9):
                                    dy, dx = t // 3 - 1, t % 3 - 1
                                    o = sb0 + dy * Wp1 + dx
                                    nc.tensor.matmul(
                                        ps[:, :N],
                                        ws["qdw_diag"][:, (t * 3 + j) * 96:(t * 3 + j + 1) * 96],
                                        qkv0[j][:, o:o + N],
                                        start=(t == 0), stop=(t == 8))
                                ps_int = ps[:, :N].rearrange("p (r w) -> p r w", w=Wp1)[:, :, 1:1 + W]
                                ch = pbr.tile([96, RC * W], F32R, tag=f"qkv_ch{j}")
                                nc.scalar.copy(
                                    ch[:, :NN].rearrange("p (r w) -> p r w", w=W), ps_int)
                                if j == 2:
                                    nc.sync.dma_start(out=v_sp[:, c0 * W:c0 * W + NN],
                                                      in_=ch[:, :NN])
                                else:
                                    qk_ch[j] = ch
                            for rr in range(c0, c0 + nr_c):
                                rl = (rr - c0) * W
                                tqk_ps = pbps.tile([W, 192], F32, tag="tqk")
                                nc.tensor.transpose(
                                    tqk_ps[:, 0:96],
                                    qk_ch[0][:, rl:rl + W].bitcast(F32),
                                    ident[:96, :96])
                                nc.tensor.transpose(
                                    tqk_ps[:, 96:192],
                                    qk_ch[1][:, rl:rl + W].bitcast(F32),
                                    ident[:96, :96])
                                qkT = pbr.tile([W, 192], F32, tag="qkT")
                                nc.scalar.copy(qkT[:], tqk_ps[:])
                                nc.tensor.matmul(g1_ps[:, 0:192], qkT[:, 0:96],
                                                 qkT[:, 0:192],
                                                 start=(rr == 0), stop=(rr == H - 1))
                                nc.tensor.matmul(g1_ps[:, 192:288], qkT[:, 96:192],
                                                 qkT[:, 96:192],
                                                 start=(rr == 0), stop=(rr == H - 1))

                # ---- gate mean -> AllReduce -> dynk ----
                gred = pers.tile([1, 1], F32)
                nc.vector.reduce_sum(gred[:], gsum[0:1, 0:NB * NGC_PER_BAND], axis=AX)
                gsc = pers.tile([1, 1], F32)
                nc.vector.tensor_scalar_mul(gsc[:], gred[:], float(CPH) / (n_cores * S))
                nc.sync.dma_start(out=cc_in[:], in_=gsc[:])
                nc.gpsimd.collective_compute(
                    "AllReduce", Alu.add, replica_groups=[list(range(n_cores))],
                    ins=[cc_in.opt()], outs=[cc_out.opt()])
                nc.sync.dma_start(out=dynk[:], in_=cc_out[:].partition_broadcast(96))

                # ---- attn block ----
                with (
                    tc.tile_pool(name="at_ps", bufs=2, space="PSUM") as atps,
                    tc.tile_pool(name="at_sb", bufs=1) as ab,
                ):
                    g1sb = ab.tile([96, 288], F32)
                    nc.scalar.copy(g1sb[:], g1_ps[:])
                    gqk = g1sb[:, 96:192]
                    idm = ident[:96, :96]
                    tq = ab.tile([96, 96], F32)
                    nc.vector.tensor_tensor(out=tq[:], in0=g1sb[:, 0:96], in1=idm, op=Alu.mult)
                    nq2 = ab.tile([96, 1], F32)
                    nc.vector.reduce_sum(nq2[:], tq[:], axis=AX)
                    tk = ab.tile([96, 96], F32)
                    nc.vector.tensor_tensor(out=tk[:], in0=g1sb[:, 192:288], in1=idm,
                                            op=Alu.mult)
                    nk2 = ab.tile([96, 1], F32)
                    nc.vector.reduce_sum(nk2[:], tk[:], axis=AX)

                    def rsqrt_clamped(nm, src):
                        sq = ab.tile([96, 1], F32, tag=nm + "sq")
                        nc.scalar.sqrt(sq[:], src[:])
                        cl = ab.tile([96, 1], F32, tag=nm + "cl")
                        nc.vector.tensor_scalar_max(cl[:], sq[:], 1e-12)
                        rvv = ab.tile([96, 1], F32, tag=nm)
                        nc.vector.reciprocal_approx_fast(rvv[:], cl[:])
                        return rvv

                    rq = rsqrt_clamped("rq", nq2)
                    rk = rsqrt_clamped("rk", nk2)
                    rqt = ab.tile([96, 1], F32)
                    nc.vector.tensor_tensor(out=rqt[:], in0=rq[:], in1=ws["tempvec"][:],
                                            op=Alu.mult)
                    asr = ab.tile([96, 96], F32)
                    nc.vector.tensor_scalar_mul(asr[:], gqk, rqt[:])
                    as_ps = atps.tile([96, 96], F32, tag="atp")
                    nc.tensor.transpose(as_ps[:], asr[:], ident[:96, :96])
                    ast = ab.tile([96, 96], F32)
                    nc.vector.tensor_scalar_mul(ast[:], as_ps[:], rk[:])
                    as2_ps = atps.tile([96, 96], F32, tag="atp")
                    nc.tensor.transpose(as2_ps[:], ast[:], ident[:96, :96])
                    as2 = ab.tile([96, 96], F32)
                    nc.scalar.copy(as2[:], as2_ps[:])
                    # mask off-head-block entries to -60
                    t60 = ab.tile([96, 96], F32)
                    nc.vector.tensor_scalar_add(t60[:], as2[:], 60.0)
                    amf = ab.tile([96, 96], F32)
                    nc.vector.tensor_tensor(out=amf[:], in0=t60[:], in1=ws["vmask"][:],
                                            op=Alu.mult)
                    nc.vector.tensor_scalar_add(amf[:], amf[:], -60.0)
                    # rank+1 over full row via pairwise is_ge
                    rnk3 = ab.tile([96, 96 * 96], F32)
                    a_i = amf[:].unsqueeze(1).broadcast_to([96, 96, 96])
                    a_d = amf[:].unsqueeze(2).broadcast_to([96, 96, 96])
                    rvw = rnk3[:].rearrange("p (i d) -> p i d", d=96)
                    nc.vector.tensor_tensor(out=rvw, in0=a_i, in1=a_d, op=Alu.is_ge)
                    rank1 = ab.tile([96, 96], F32)
                    nc.vector.reduce_sum(rank1[:].unsqueeze(2), rvw, axis=AX)
                    sel = ab.tile([96, 96], F32)
                    nc.vector.tensor_tensor(out=sel[:], in0=rank1[:],
                                            in1=dynk[:].broadcast_to([96, 96]), op=Alu.is_le)
                    am = ab.tile([96, 96], F32)
                    t60b = ab.tile([96, 96], F32)
                    nc.vector.tensor_scalar_add(t60b[:], amf[:], 60.0)
                    nc.vector.tensor_tensor(out=am[:], in0=t60b[:], in1=sel[:], op=Alu.mult)
                    nc.vector.tensor_scalar_add(am[:], am[:], -60.0)
                    mx = ab.tile([96, 1], F32)
                    nc.vector.reduce_max(mx[:], am[:], axis=AX)
                    nmx = ab.tile([96, 1], F32)
                    nc.vector.tensor_scalar_mul(nmx[:], mx[:], -1.0)
                    ex = ab.tile([96, 96], F32)
                    nc.scalar.activation(ex[:], am[:], Act.Exp, bias=nmx[:])
                    sme = ab.tile([96, 1], F32)
                    nc.vector.reduce_sum(sme[:], ex[:], axis=AX)
                    rsm = ab.tile([96, 1], F32)
                    nc.vector.reciprocal_approx_fast(rsm[:], sme[:])
                    probs = ab.tile([96, 96], F32)
                    nc.vector.tensor_scalar_mul(probs[:], ex[:], rsm[:])
                    pt_ps = atps.tile([96, 96], F32, tag="atp2")
                    nc.tensor.transpose(pt_ps[:], probs[:], ident[:96, :96])
                    nc.scalar.copy(probsT[:], pt_ps[:])

            # ================= PHASE B5 =================
            with (
                tc.tile_pool(name="b5_rot", bufs=3) as b5r,
                tc.tile_pool(name="b5_ps", bufs=1, space="PSUM") as b5ps,
            ):
                for ci in range(NSC):
                    o0 = ci * 512
                    NN = min(512, S - o0)
                    vch = b5r.tile([96, 512], F32R, tag="vch")
                    nc.sync.dma_start(out=vch[:, :NN], in_=v_sp[:, o0:o0 + NN])
                    av_ps = b5ps.tile([96, 512], F32, tag="avps", bufs=2)
                    nc.tensor.matmul(av_ps[:, :NN], probsT[:], vch[:, :NN],
                                     start=True, stop=True)
                    avs = b5r.tile([96, 512], F32R, tag="avs")
                    nc.scalar.activation(avs[:, :NN], av_ps[:, :NN], Act.Copy,
                                         scale=attn_scale)
                    x2ch = b5r.tile([96, 512], F32R, tag="x2ch")
                    nc.sync.dma_start(out=x2ch[:, :NN], in_=yn2_sp[:, o0:o0 + NN])
                    xpch = [b5r.tile([96, 512], F32R, tag=f"xp{cg}", name=f"xp{cg}") for cg in range(2)]
                    xsq = [b5r.tile([96, 512], F32R, tag=f"xs{cg}", name=f"xs{cg}") for cg in range(2)]
                    for cg in range(2):
                        xcch = b5r.tile([96, 512], F32R, tag=f"xcc{cg}")
                        nc.sync.dma_start(out=xcch[:, :NN], in_=xc_sp[cg][:, o0:o0 + NN])
                        pj_ps = b5ps.tile([96, 512], F32, tag=f"pjps{cg}", bufs=2)
                        nc.tensor.matmul(pj_ps[:, :NN],
                                         ws["proj1T"][:, cg * 96:(cg + 1) * 96],
                                         avs[:, :NN], start=True, stop=False)
                        nc.tensor.matmul(pj_ps[:, :NN],
                                         ws["proj2T"][:, cg * 96:(cg + 1) * 96],
                                         x2ch[:, :NN], start=False, stop=True)
                        nc.vector.scalar_tensor_tensor(
                            out=xpch[cg][:, :NN], in0=pj_ps[:, :NN],
                            scalar=ws["proj_bias"][:, cg:cg + 1], in1=xcch[:, :NN],
                            op0=Alu.add, op1=Alu.add)
                        nc.sync.dma_start(out=xcp_sp[cg][:, o0:o0 + NN],
                                          in_=xpch[cg][:, :NN])
                        nc.scalar.square(xsq[cg][:, :NN], xpch[cg][:, :NN])
                    # LN2 applied here; spill the normalized activations so the
                    # C-phase loop needs no stats broadcast at all
                    mu_ps = b5ps.tile([128, 512], F32, tag="mu2ps", bufs=1)
                    m2_ps = b5ps.tile([128, 512], F32, tag="m22ps", bufs=1)
                    for cg in range(2):
                        nc.tensor.matmul(mu_ps[:, :NN], ws["ones_st"], xpch[cg][:, :NN],
                                         start=(cg == 0), stop=(cg == 1))
                        nc.tensor.matmul(m2_ps[:, :NN], ws["ones_st"], xsq[cg][:, :NN],
                                         start=(cg == 0), stop=(cg == 1))
                    musq = b5r.tile([128, 512], F32, tag="musq2")
                    nc.scalar.square(musq[:, :NN], mu_ps[:, :NN])
                    var = b5r.tile([128, 512], F32, tag="var2")
                    nc.vector.tensor_tensor(out=var[:, :NN], in0=m2_ps[:, :NN],
                                            in1=musq[:, :NN], op=Alu.subtract)
                    sd = b5r.tile([128, 512], F32, tag="sd2")
                    nc.scalar.activation(sd[:, :NN], var[:, :NN], Act.Sqrt, bias=ws["epsv"])
                    rstd = b5r.tile([128, 512], F32, tag="rstd2")
                    nc.vector.reciprocal_approx_fast(rstd[:, :NN], sd[:, :NN])
                    for cg in range(2):
                        td = b5r.tile([96, 512], F32, tag=f"td{cg}")
                        nc.vector.tensor_tensor(out=td[:, :NN], in0=xpch[cg][:, :NN],
                                                in1=mu_ps[:96, :NN], op=Alu.subtract)
                        ynn = b5r.tile([96, 512], F32R, tag=f"ynn{cg}")
                        nc.vector.tensor_tensor(out=ynn[:, :NN], in0=td[:, :NN],
                                                in1=rstd[:96, :NN], op=Alu.mult)
                        nc.sync.dma_start(out=ynn_sp[cg][:, o0:o0 + NN],
                                          in_=ynn[:, :NN])

            _wpab_cm.__exit__(None, None, None)
            # ================= PHASE C =================
            _wpc_cm = tc.tile_pool(name="wpC", bufs=1)
            wpc = _wpc_cm.__enter__()
            _load_w(wpc, C_ONLY_W)
            with tc.tile_pool(name="c_v0", bufs=1) as cv0:
                v0t1 = cv0.tile([128, P3], F32R)
                v0t2 = cv0.tile([128, P3], F32R)
                with (
                    tc.tile_pool(name="c1_rot", bufs=2) as c1r,
                    tc.tile_pool(name="c2_rot", bufs=2) as c2r,
                    tc.tile_pool(name="c_ps", bufs=1, space="PSUM") as cps,
                ):
                    # pad cells must hold -t1/s1 so the bn-folded depthwise
                    # reads zeros in v0_bn space at image borders; only the
                    # border strips are ever read as pad (interior is written
                    # by the fc1 stage below), so skip the full-buffer memset
                    for v0t, pv in ((v0t1, ws["padv1"]), (v0t2, ws["padv2"])):
                        strips = [
                            v0t[:, 0:SLOP + 3 * Wp3],
                            v0t[:, SLOP + (H + 3) * Wp3:P3],
                        ]
                        mid = v0t[:, SLOP + 3 * Wp3:SLOP + (H + 3) * Wp3] \
                            .rearrange("p (r w) -> p r w", w=Wp3)
                        strips.append(mid[:, :, 0:4])
                        strips.append(mid[:, :, 131:134])
                        for st in strips:
                            nc.vector.memset(st.bitcast(F32), 0.0)
                            nc.vector.tensor_scalar_add(st, st, pv)

                    def c1_body(ci):
                        c0 = ci * RC
                        nr_c = min(RC, H - c0)
                        NN = nr_c * W
                        o0 = c0 * W
                        yn2t = [c1r.tile([96, RC * W], F32R, tag=f"cy{cg}", name=f"cy{cg}") for cg in range(2)]
                        for cg in range(2):
                            nc.sync.dma_start(out=yn2t[cg][:, :NN],
                                              in_=ynn_sp[cg][:, o0:o0 + NN])
                        sb0c = pd3(3 + c0)
                        # g0: channels 0:64 -> vg0/ug0
                        fg0 = cps.tile([64, RC * W], F32, tag="fg0", bufs=1)
                        for cg in range(2):
                            nc.tensor.matmul(fg0[:, :NN],
                                             ws["fc1T"][:, cg * 64:(cg + 1) * 64],
                                             yn2t[cg][:, :NN],
                                             start=(cg == 0), stop=(cg == 1))
                        fg2 = cps.tile([64, RC * W], F32, tag="fg2", bufs=1)
                        for cg in range(2):
                            nc.tensor.matmul(fg2[:, :NN],
                                             ws["fc1T"][:, 384 + cg * 64:448 + cg * 64],
                                             yn2t[cg][:, :NN],
                                             start=(cg == 0), stop=(cg == 1))
                        vg0 = c1r.tile([64, RC * W], F32R, tag="vg0")
                        nc.scalar.activation(vg0[:, :NN], fg0[:, :NN], Act.Gelu,
                                             bias=ws["fc1_bias"][0:64, 0:1])
                        ug0 = c1r.tile([64, RC * W], F32R, tag="ug0")
                        nc.scalar.activation(ug0[:, :NN], vg0[:, :NN], Act.Gelu,
                                             bias=ws["g0_bias"], scale=ws["g0_scale"])
                        for r in range(nr_c):
                            d0 = pd3(3 + c0 + r) + 3
                            nc.sync.dma_start(out=vg0_sp[:, d0:d0 + W],
                                              in_=vg0[:, r * W:(r + 1) * W])
                            nc.sync.dma_start(out=ug0_sp[:, d0:d0 + W],
                                              in_=ug0[:, r * W:(r + 1) * W])
                        # g1: channels 64:192 -> v0t1, one strided gelu
                        fg1 = cps.tile([128, RC * W], F32, tag="fg1", bufs=1)
                        for cg in range(2):
                            nc.tensor.matmul(fg1[:, :NN],
                                             ws["fc1T"][:, 128 + cg * 128:256 + cg * 128],
                                             yn2t[cg][:, :NN],
                                             start=(cg == 0), stop=(cg == 1))
                        dv1 = v0t1[:, sb0c:sb0c + nr_c * Wp3].rearrange(
                            "p (r w) -> p r w", w=Wp3)[:, :, 3:3 + W]
                        nc.scalar.activation(
                            dv1, fg1[:, :NN].rearrange("p (r w) -> p r w", w=W),
                            Act.Gelu, bias=ws["fc1_bias"][:, 1:2])
                        # g2: channels 192:256 -> v0t2 halves (second shifted +1)
                        dv2a = v0t2[0:64, sb0c:sb0c + nr_c * Wp3].rearrange(
                            "p (r w) -> p r w", w=Wp3)[:, :, 3:3 + W]
                        nc.scalar.activation(
                            dv2a, fg2[:, :NN].rearrange("p (r w) -> p r w", w=W),
                            Act.Gelu, bias=ws["fc1_bias"][64:128, 0:1])
                        dv2b = v0t2[64:128, sb0c:sb0c + nr_c * Wp3].rearrange(
                            "p (r w) -> p r w", w=Wp3)[:, :, 4:4 + W]
                        nc.scalar.activation(
                            dv2b, fg2[:, :NN].rearrange("p (r w) -> p r w", w=W),
                            Act.Gelu, bias=ws["fc1_bias"][64:128, 0:1])

                    def c2_body(ci):
                        c0 = ci * RC
                        nr_c = min(RC, H - c0)
                        N = nr_c * Wp3
                        NN = nr_c * W
                        sb0 = pd3(3 + c0)
                        ps_a = cps.tile([128, RC * Wp3], F32, tag="psa", bufs=2)
                        for t in range(25):
                            dy, dx = t // 5 - 2, t % 5 - 2
                            o = sb0 + dy * Wp3 + dx
                            nc.tensor.matmul(ps_a[:, :N],
                                             ws["pair_diag"][:, t * 128:(t + 1) * 128],
                                             v0t1[:, o:o + N],
                                             start=(t == 0), stop=(t == 24))
                        ps_b = cps.tile([64, RC * Wp3], F32, tag="psb", bufs=1)
                        for i, (dy, dxa, hasb) in enumerate(dw3_passes):
                            o = sb0 + dy * Wp3 + dxa
                            nc.tensor.matmul(ps_b[:, :N],
                                             ws["dw3_diag"][:, i * 64:(i + 1) * 64],
                                             v0t2[:, o:o + N],
                                             start=(i == 0), stop=(i == len(dw3_passes) - 1))

                        def inner(ap_flat, lo, hi):
                            # interior view of a PSUM chunk (starts at free 0)
                            return ap_flat[lo:hi, :N].rearrange(
                                "p (r w) -> p r w", w=Wp3)[:, :, 3:3 + W]

                        def inner_v0(ap_flat, lo, hi):
                            # interior view of the padded v0 buffers at this chunk
                            return ap_flat[lo:hi, sb0:sb0 + N].rearrange(
                                "p (r w) -> p r w", w=Wp3)[:, :, 3:3 + W]

                        ug_a = c2r.tile([128, RC * W], F32R, tag="uga")
                        ug_b = c2r.tile([128, RC * W], F32R, tag="ugb")
                        vb_a = c2r.tile([128, RC * W], F32R, tag="vba")
                        vb_b = c2r.tile([128, RC * W], F32R, tag="vbb")
                        g0v = c2r.tile([64, RC * W], F32R, tag="g0v")
                        src3 = vg0_sp[:, sb0:sb0 + N].rearrange(
                            "p (r w) -> p r w", w=Wp3)[:, :, 3:3 + W]
                        nc.sync.dma_start(
                            out=g0v[:, :NN].rearrange("p (r w) -> p r w", w=W), in_=src3)
                        usrc3 = ug0_sp[:, sb0:sb0 + N].rearrange(
                            "p (r w) -> p r w", w=Wp3)[:, :, 3:3 + W]
                        nc.sync.dma_start(
                            out=ug_a[0:64, :NN].rearrange("p (r w) -> p r w", w=W), in_=usrc3)
                        nc.scalar.activation(
                            ug_a[64:128, :NN].rearrange("p (r w) -> p r w", w=W),
                            inner(ps_a, 0, 64), Act.Gelu, bias=ws["pair_bias"][0:64])
                        nc.scalar.activation(
                            ug_b[0:64, :NN].rearrange("p (r w) -> p r w", w=W),
                            inner(ps_a, 64, 128), Act.Gelu, bias=ws["pair_bias"][64:128])
                        nc.scalar.activation(
                            ug_b[64:128, :NN].rearrange("p (r w) -> p r w", w=W),
                            inner(ps_b, 0, 64), Act.Gelu, bias=ws["dw3_bias"])
                        nc.vector.tensor_scalar(out=vb_a[0:64, :NN], in0=g0v[:, :NN],
                                                scalar1=ws["s1a"][0:64],
                                                scalar2=ws["t1a"][0:64],
                                                op0=Alu.mult, op1=Alu.add)
                        nc.vector.tensor_scalar(out=vb_a[64:128, :NN],
                                                in0=inner_v0(v0t1, 0, 64),
                                                scalar1=ws["s1a"][64:128],
                                                scalar2=ws["t1a"][64:128],
                                                op0=Alu.mult, op1=Alu.add)
                        nc.vector.tensor_scalar(out=vb_b[0:64, :NN],
                                                in0=inner_v0(v0t1, 64, 128),
                                                scalar1=ws["s1b"][0:64],
                                                scalar2=ws["t1b"][0:64],
                                                op0=Alu.mult, op1=Alu.add)
                        nc.vector.tensor_scalar(out=vb_b[64:128, :NN],
                                                in0=inner_v0(v0t2, 0, 64),
                                                scalar1=ws["s1b"][64:128],
                                                scalar2=ws["t1b"][64:128],
                                                op0=Alu.mult, op1=Alu.add)
                        # z1 = ug * vb computed in place into the vb tiles
                        nc.vector.tensor_tensor(out=vb_a[:, :NN], in0=ug_a[:, :NN],
                                                in1=vb_a[:, :NN], op=Alu.mult)
                        nc.vector.tensor_tensor(out=vb_b[:, :NN], in0=ug_b[:, :NN],
                                                in1=vb_b[:, :NN], op=Alu.mult)
                        occ = {}
                        tpps = {}
                        for cg in range(2):
                            # ops [96, 0:384] and the output-transpose psum
                            # [128, 384:480] share one bank
                            cmb = cps.tile([128, RC * W + 96], F32, tag=f"cmb{cg}",
                                           bufs=1)
                            ops = cmb[0:96, 0:RC * W]
                            ops = cmb[0:96, 0:NN]
                            tpps[cg] = cmb[:, RC * W:RC * W + 96]
                            nc.tensor.matmul(ops,
                                             ws["fc2aT"][:, (cg * 2) * 96:(cg * 2 + 1) * 96],
                                             vb_a[:, :NN], start=True, stop=False)
                            nc.tensor.matmul(ops,
                                             ws["fc2aT"][:, (cg * 2 + 1) * 96:(cg * 2 + 2) * 96],
                                             vb_b[:, :NN], start=False, stop=False)
                            nc.tensor.matmul(ops,
                                             ws["fc2bT_g0"][:, cg * 96:(cg + 1) * 96],
                                             g0v[:, :NN], start=False, stop=False)
                            opsv = ops.rearrange("p (r w) -> p r w", w=W)
                            nc.tensor.matmul(opsv,
                                             ws["fc2bT_g12"][:, cg * 96:(cg + 1) * 96],
                                             inner_v0(v0t1, 0, 128), start=False, stop=False)
                            nc.tensor.matmul(opsv,
                                             ws["fc2bT_g3"][:, cg * 96:(cg + 1) * 96],
                                             inner_v0(v0t2, 0, 64), start=False, stop=True)
                            xrch = c2r.tile([96, RC * W], F32R, tag=f"xr{cg}", bufs=1)
                            nc.sync.dma_start(out=xrch[:, :NN],
                                              in_=xcp_sp[cg][:, c0 * W:c0 * W + NN])
                            ob = c2r.tile([96, RC * W], F32, tag=f"ob{cg}", bufs=1)
                            nc.vector.tensor_scalar(out=ob[:, :NN], in0=ops,
                                                    scalar1=ws["s3v"][:, cg:cg + 1],
                                                    scalar2=ws["out_bias"][:, cg:cg + 1],
                                                    op0=Alu.mult, op1=Alu.add)
                            nc.vector.tensor_tensor(out=ob[:, :NN], in0=ob[:, :NN],
                                                    in1=xrch[:, :NN], op=Alu.add)
                            occ[cg] = ob
                        # transpose to pixel-major [W, EMBED] per image row so the
                        # output DMA writes contiguous 768B lines instead of a
                        # 4B-per-element scatter
                        outT = c2r.tile([W, RC * EMBED], F32, tag="outT", bufs=1)
                        for r in range(nr_c):
                            for cg in range(2):
                                nc.tensor.transpose(
                                    tpps[cg], occ[cg][:, r * W:(r + 1) * W],
                                    ws["ident"][:96, :96])
                                nc.scalar.copy(
                                    outT[:, r * EMBED + cg * 96:r * EMBED + (cg + 1) * 96],
                                    tpps[cg])
                        for r in range(nr_c):
                            nc.sync.dma_start(
                                out=out_t[(c0 + r) * W:(c0 + r + 1) * W, :],
                                in_=outT[:, r * EMBED:(r + 1) * EMBED])

                    # interleave: fc1/gelu of chunk it overlaps the
                    # TensorE-bound depthwise/fc2 of chunk it-2
                    for it in range(NCH + 2):
                        if it < NCH:
                            c1_body(it)
                        if it >= 2:
                            c2_body(it - 2)
            _wpc_cm.__exit__(None, None, None)
    return out_t.name


# ----------------------------------------------------------------------------
# host entry
# ----------------------------------------------------------------------------

_CACHE = {}


def make_program(H, W, n_cores, attn_scale, dw3_passes):
    key = (H, W, n_cores, round(attn_scale, 9))
    if key in _CACHE:
        return _CACHE[key]
    nc = bacc.Bacc("TRN2", target_bir_lowering=False, debug=False, num_devices=n_cores)
    out_name = build(nc, H, W, n_cores, attn_scale, dw3_passes)
    nc.compile()
    _CACHE[key] = (nc, out_name)
    return nc, out_name


def make_in_maps(inputs):
    x = np.asarray(inputs["x"], np.float32)
    B = x.shape[0]
    wdict = _prep_weights({k: np.asarray(v) for k, v in inputs.items()})
    base = {}
    for k, (shp, d) in WSPEC.items():
        base["w_" + k] = wdict[k][0].reshape(shp)
    in_maps = []
    for b in range(B):
        m = dict(base)
        m["x"] = np.ascontiguousarray(x[b])
        in_maps.append(m)
    return in_maps, wdict


def kernel(**inputs):
    x = np.asarray(inputs["x"], np.float32)
    B, H, W, C = x.shape
    in_maps, wdict = make_in_maps(inputs)
    nc, out_name = make_program(H, W, B, wdict["_attn_scale"][0],
                                wdict["_dw3_passes"][0])
    res = bass_utils.run_bass_kernel_spmd(nc, in_maps, core_ids=list(range(B)))
    return np.stack([res.results[b][out_name].reshape(H, W, C) for b in range(B)])



# revision 58
# speedup vs baseline: 1.1530x; 1.0002x over previous
"""Trainium2 Bass kernel for nn_Block_87351044866235 (sparse_attention).

Data-parallel over batch: 8 samples -> 8 NeuronCores. Channel-major
layout [C, H*W] on chip; depthwise convs as diagonal fp32r matmuls on
TensorE; 1x1 convs as fp32r matmuls; LN stats via ones-matmuls; q/k gram
via per-row TensorE transposes + fp32 matmuls; dynamic-k gate mean via a
scalar AllReduce; LN2 applied in the B5 stage (normalized activations
spilled); fc1/gelu stage interleaved with the TensorE-bound depthwise/
fc2 stage; output transposed on TensorE to pixel-major so the final DMA
writes contiguous 768B lines.
"""
import sys, os

for _p in ("/opt/trn_rl_repo", "/root/.axon_site/_ro/trn_rl_repo"):
    if os.path.isdir(_p) and _p not in sys.path:
        sys.path.append(_p)

import numpy as np
import concourse.bass as bass
import concourse.bacc as bacc
import concourse.tile as tile
from concourse import mybir
from concourse import bass_utils

try:
    from concourse import tile_utils as _tu
    _tu.max_sbuf_usage = 208 * 1024
except Exception:
    pass

dt = mybir.dt
Alu = mybir.AluOpType
Act = mybir.ActivationFunctionType
AX = mybir.AxisListType.X

EMBED, PDIM, HEADS, HID = 192, 96, 8, 256
CPH = PDIM // HEADS  # 12
SLOP = 8
RC = 3    # conv output rows per chunk
BR = 12   # rows per band


F32, F32R, BF16 = dt.float32, dt.float32r, dt.bfloat16


def _ceil(a, b):
    return (a + b - 1) // b


# ----------------------------------------------------------------------------
# host-side weight prep: everything 2D [partitions, free]
# ----------------------------------------------------------------------------

def _prep_weights(p):
    w = {}
    f32r = lambda a: (np.ascontiguousarray(a, np.float32), F32R)
    f32 = lambda a: (np.ascontiguousarray(a, np.float32), F32)
    eps_bn = 1e-5

    w["ident"] = f32(np.eye(128, dtype=np.float32))
    w["identr"] = f32r(np.eye(128, dtype=np.float32))

    # pos depthwise diag: [96, (t*2+cg)*96]
    pw = p["pos_w"][:, 0]  # [192,3,3]
    pos_d = np.zeros((96, 18 * 96), np.float32)
    for t in range(9):
        dy, dx = t // 3 - 1, t % 3 - 1
        for cg in range(2):
            pos_d[:, (t * 2 + cg) * 96:(t * 2 + cg + 1) * 96] = \
                np.diag(pw[cg * 96:(cg + 1) * 96, dy + 1, dx + 1])
    w["pos_diag"] = f32r(pos_d)
    w["pos_b"] = f32(p["pos_b"].reshape(2, 96).T)  # [96, 2]

    g1v, b1v = p["ln1_g"], p["ln1_b"]
    qw = p["qkv_w"][:, :, 0, 0]  # [288, 96]
    qw_eff = qw * g1v[None, :96]
    w["qkv_wT"] = f32r(np.concatenate(
        [qw_eff[j * 96:(j + 1) * 96].T for j in range(3)], axis=1))  # [96, 3*96]
    w["qkv_bias"] = f32((qw @ b1v[:96]).reshape(3, 96).T)  # [96, 3]

    qdw = p["qkv_dw_w"][:, 0]  # [288,3,3]
    qdw_d = np.zeros((96, 27 * 96), np.float32)
    for t in range(9):
        dy, dx = t // 3 - 1, t % 3 - 1
        for j in range(3):
            qdw_d[:, (t * 3 + j) * 96:(t * 3 + j + 1) * 96] = \
                np.diag(qdw[j * 96:(j + 1) * 96, dy + 1, dx + 1])
    w["qdw_diag"] = f32r(qdw_d)

    gw1 = p["gate_w1"][:, :, 0, 0]  # [96, 192]
    gw1_eff = gw1 * g1v[None, :]
    w["gate_w1T"] = f32r(np.concatenate(
        [gw1_eff[:, cg * 96:(cg + 1) * 96].T for cg in range(2)], axis=1))  # [96, 192]
    w["gate_b1"] = f32((p["gate_b1"] + gw1 @ b1v).reshape(96, 1))
    w["gate_w2T"] = f32r(p["gate_w2"][:, :, 0, 0].T.copy())  # [96,1]
    w["gate_b2"] = f32(p["gate_b2"].reshape(1, 1))

    pj = p["proj_w"][:, :, 0, 0]
    pj1, pj2 = pj[:, :96], pj[:, 96:] * g1v[None, 96:]
    w["proj1T"] = f32r(np.concatenate(
        [pj1[cg * 96:(cg + 1) * 96].T for cg in range(2)], axis=1))  # [96, 192]
    w["proj2T"] = f32r(np.concatenate(
        [pj2[cg * 96:(cg + 1) * 96].T for cg in range(2)], axis=1))
    w["proj_bias"] = f32((pj[:, 96:] @ b1v[96:]).reshape(2, 96).T)  # [96, 2]

    attn_scale = float(p["attn1"][0] + p["attn2"][0] + p["attn3"][0] + p["attn4"][0])
    w["_attn_scale"] = (attn_scale, None)
    w["tempvec"] = f32(np.repeat(p["temperature"].reshape(HEADS), CPH).reshape(96, 1))

    g2v, b2v = p["ln2_g"], p["ln2_b"]
    f1 = p["fc1_w"][:, :, 0, 0]  # [256, 192]
    f1_eff = f1 * g2v[None, :]
    # channel groups 64|128|64 so each group's gelu lands in one strided op:
    # g0 -> vg0 (ch 0:64), g1 -> v0t1 (ch 64:192), g2 -> v0t2 (ch 192:256)
    fc1 = np.zeros((96, 512), np.float32)
    fc1[:, 0:64] = f1_eff[0:64, 0:96].T
    fc1[:, 64:128] = f1_eff[0:64, 96:192].T
    fc1[:, 128:256] = f1_eff[64:192, 0:96].T
    fc1[:, 256:384] = f1_eff[64:192, 96:192].T
    fc1[:, 384:448] = f1_eff[192:256, 0:96].T
    fc1[:, 448:512] = f1_eff[192:256, 96:192].T
    w["fc1T"] = f32r(fc1)
    fb = f1 @ b2v
    fbias = np.zeros((128, 2), np.float32)
    fbias[0:64, 0] = fb[0:64]
    fbias[64:128, 0] = fb[192:256]
    fbias[:, 1] = fb[64:192]
    w["fc1_bias"] = f32(fbias)

    s1 = p["bn1_g"] / np.sqrt(p["bn1_v"] + eps_bn)
    t1 = p["bn1_b"] - p["bn1_m"] * s1
    s2 = p["bn2_g"] / np.sqrt(p["bn2_v"] + eps_bn)
    t2 = p["bn2_b"] - p["bn2_m"] * s2
    s3 = p["bn3_g"] / np.sqrt(p["bn3_v"] + eps_bn)
    t3 = p["bn3_b"] - p["bn3_m"] * s3

    dw1w, dw2w, dw3w = p["dw1_w"][:, 0], p["dw2_w"][:, 0], p["dw3_w"][:, 0]
    dw1b, dw2b, dw3b = p["dw1_b"], p["dw2_b"], p["dw3_b"]
    s1g = [s1[i * 64:(i + 1) * 64] for i in range(4)]
    t1g = [t1[i * 64:(i + 1) * 64] for i in range(4)]

    def pair_tap_diag(t):
        dy, dx = t // 5 - 2, t % 5 - 2
        v = np.zeros(128, np.float32)
        d2 = dw2w[:, dy + 2, dx + 2] * s1g[2]
        if dy == 0 and dx == 0:
            d2 = d2 + s1g[2]
        v[64:] = d2
        if -1 <= dy <= 1 and -1 <= dx <= 1:
            d1 = dw1w[:, dy + 1, dx + 1] * s1g[1]
            if dy == 0 and dx == 0:
                d1 = d1 + s1g[1]
            v[:64] = d1
        return v

    pair_d = np.zeros((128, 25 * 128), np.float32)
    for t in range(25):
        pair_d[:, t * 128:(t + 1) * 128] = np.diag(pair_tap_diag(t))
    w["pair_diag"] = f32r(pair_d)
    bc1 = t1g[1] * dw1w.sum((1, 2)) + dw1b + t1g[1]
    bc2 = t1g[2] * dw2w.sum((1, 2)) + dw2b + t1g[2]
    w["pair_bias"] = f32(np.concatenate([bc1, bc2]).reshape(128, 1))

    # rows 64:128 of v0t2 hold the same data stored shifted +1, so a read at
    # AP offset (dy, dxa) yields tap (dy, dxa-1) for those rows.
    dw3_passes = []
    for dy in range(-3, 4):
        for dxa in (-2, 0, 2):
            dw3_passes.append((dy, dxa, True))
        dw3_passes.append((dy, 3, False))
    dw3_d = np.zeros((128, len(dw3_passes) * 64), np.float32)
    for i, (dy, dxa, hasb) in enumerate(dw3_passes):
        wa = dw3w[:, dy + 3, dxa + 3] * s1g[3]
        if dy == 0 and dxa == 0:
            wa = wa + s1g[3]
        dw3_d[:64, i * 64:(i + 1) * 64] = np.diag(wa)
        if hasb:
            wb = dw3w[:, dy + 3, dxa - 1 + 3] * s1g[3]
            if dy == 0 and dxa - 1 == 0:
                wb = wb + s1g[3]
            dw3_d[64:, i * 64:(i + 1) * 64] = np.diag(wb)
    w["dw3_diag"] = f32r(dw3_d)
    w["_dw3_passes"] = (dw3_passes, None)
    w["dw3_bias"] = f32((t1g[3] * dw3w.sum((1, 2)) + dw3b + t1g[3]).reshape(64, 1))

    d0w, d0b = p["dw0_w"][:, 0, 0, 0], p["dw0_b"]
    w["g0_scale"] = f32(((d0w + 1.0) * s1g[0]).reshape(64, 1))
    w["g0_bias"] = f32(((d0w + 1.0) * t1g[0] + d0b).reshape(64, 1))

    f2 = p["fc2_w"][:, :, 0, 0]  # [192, 256]
    f2a = f2 * s2[None, :]
    f2b = f2 * (t2 * s1)[None, :]
    cstv = f2 @ (t2 * t1)
    fc2a = np.zeros((128, 4 * 96), np.float32)
    for cg in range(2):
        for kg in range(2):
            fc2a[:, (cg * 2 + kg) * 96:(cg * 2 + kg + 1) * 96] = \
                f2a[cg * 96:(cg + 1) * 96, kg * 128:(kg + 1) * 128].T
    w["fc2aT"] = f32r(fc2a)
    w["fc2bT_g0"] = f32r(np.concatenate(
        [f2b[cg * 96:(cg + 1) * 96, 0:64].T for cg in range(2)], axis=1))    # [64, 192]
    w["fc2bT_g12"] = f32r(np.concatenate(
        [f2b[cg * 96:(cg + 1) * 96, 64:192].T for cg in range(2)], axis=1))  # [128, 192]
    w["fc2bT_g3"] = f32r(np.concatenate(
        [f2b[cg * 96:(cg + 1) * 96, 192:256].T for cg in range(2)], axis=1))  # [64, 192]
    w["s3v"] = f32(np.stack([s3[:96], s3[96:]], axis=1))          # [96, 2]
    w["out_bias"] = f32(np.stack([(s3 * 0 + t3 + s3 * cstv * 0)[:96], (t3)[96:]], axis=1))
    # careful: out = s3*(psum + cst) + t3 + xc' ; psum excludes cst, so bias = s3*cst + t3
    ob = s3 * cstv + t3
    w["out_bias"] = f32(np.stack([ob[:96], ob[96:]], axis=1))     # [96, 2]

    sg = np.where(s1 == 0, 1.0, s1)
    padv = -t1 / sg
    w["padv1"] = f32(np.concatenate([padv[64:128], padv[128:192]]).reshape(128, 1))
    w["padv2"] = f32(np.concatenate([padv[192:256], padv[192:256]]).reshape(128, 1))
    w["s1a"] = f32(s1[:128].reshape(128, 1))
    w["s1b"] = f32(s1[128:].reshape(128, 1))
    w["t1a"] = f32(t1[:128].reshape(128, 1))
    w["t1b"] = f32(t1[128:].reshape(128, 1))

    w["ones_st"] = f32r(np.full((96, 128), 1.0 / EMBED, np.float32))
    w["epsv"] = f32(np.full((128, 1), 1e-6, np.float32))
    vm = np.zeros((96, 96), np.float32)
    for h in range(HEADS):
        vm[h * CPH:(h + 1) * CPH, h * CPH:(h + 1) * CPH] = 1.0
    w["vmask"] = f32(vm)
    return w


WSPEC = {
    "ident": ([128, 128], F32), "identr": ([128, 128], F32R),
    "pos_diag": ([96, 18 * 96], F32R),
    "pos_b": ([96, 2], F32), "qkv_wT": ([96, 3 * 96], F32R),
    "qkv_bias": ([96, 3], F32), "qdw_diag": ([96, 27 * 96], F32R),
    "gate_w1T": ([96, 192], F32R), "gate_b1": ([96, 1], F32),
    "gate_w2T": ([96, 1], F32R), "gate_b2": ([1, 1], F32),
    "proj1T": ([96, 192], F32R), "proj2T": ([96, 192], F32R),
    "proj_bias": ([96, 2], F32), "tempvec": ([96, 1], F32),
    "fc1T": ([96, 4 * 128], F32R), "fc1_bias": ([128, 2], F32),
    "pair_diag": ([128, 25 * 128], F32R), "pair_bias": ([128, 1], F32),
    "dw3_diag": ([128, 28 * 64], F32R), "dw3_bias": ([64, 1], F32),
    "g0_scale": ([64, 1], F32), "g0_bias": ([64, 1], F32),
    "fc2aT": ([128, 4 * 96], F32R), "fc2bT_g0": ([64, 192], F32R),
    "fc2bT_g12": ([128, 192], F32R), "fc2bT_g3": ([64, 192], F32R),
    "s3v": ([96, 2], F32), "out_bias": ([96, 2], F32),
    "padv1": ([128, 1], F32),
    "padv2": ([128, 1], F32),
    "s1a": ([128, 1], F32), "s1b": ([128, 1], F32),
    "t1a": ([128, 1], F32), "t1b": ([128, 1], F32),
    "ones_st": ([96, 128], F32R),
    "epsv": ([128, 1], F32),
    "vmask": ([96, 96], F32),
}


# ----------------------------------------------------------------------------
# device kernel
# ----------------------------------------------------------------------------

def build(nc, H, W, n_cores, attn_scale, dw3_passes):
    S = H * W
    Wp1 = W + 2
    P1B = (BR + 2) * Wp1 + 2 * SLOP   # band buffer (pad1)
    Wp3, Hp3 = W + 6, H + 6
    P3 = Hp3 * Wp3 + 2 * SLOP
    NCH = _ceil(H, RC)
    NB = _ceil(H, BR)
    NSC = _ceil(S, 512)
    GCH = 512 // W                    # gate chunk rows (512 cols)
    NGC_PER_BAND = _ceil(BR, GCH)

    x_t = nc.dram_tensor("x", [H, W, EMBED], F32, kind="ExternalInput")
    out_t = nc.dram_tensor("out", [S, EMBED], F32, kind="ExternalOutput")
    wt = {k: nc.dram_tensor("w_" + k, shp, d, kind="ExternalInput")
          for k, (shp, d) in WSPEC.items()}

    def pd3(r):
        return SLOP + r * Wp3

    with tile.TileContext(nc) as tc:
        C_ONLY_W = ['fc1T', 'fc1_bias', 'pair_diag', 'pair_bias', 'dw3_diag', 'dw3_bias', 'g0_scale', 'g0_bias', 'fc2aT', 'fc2bT_g0', 'fc2bT_g12', 'fc2bT_g3', 's3v', 'out_bias', 's1a', 's1b', 't1a', 't1b', 'padv1', 'padv2']
        with (
            tc.tile_pool(name="dram", bufs=1, space="DRAM") as dram,
            tc.tile_pool(name="persist", bufs=1) as pers,
        ):
            ws = {}

            def _load_w(pool, names):
                for k in names:
                    shp, d = WSPEC[k]
                    tl = pool.tile(shp, d, tag="w_" + k, name="w_" + k)
                    nc.sync.dma_start(out=tl[:], in_=wt[k][:])
                    ws[k] = tl


            yn1_sp = dram.tile([96, S], F32R)
            yn2_sp = dram.tile([96, S], F32R)
            xc_sp = [dram.tile([96, S], F32R, name=f"xc_sp{i}") for i in range(2)]
            v_sp = dram.tile([96, S], F32R)
            xcp_sp = [dram.tile([96, S], F32R, name=f"xcp_sp{i}") for i in range(2)]
            ynn_sp = [dram.tile([96, S], F32R, name=f"ynn_sp{i}") for i in range(2)]
            vg0_sp = dram.tile([64, P3], F32R)
            ug0_sp = dram.tile([64, P3], F32R)
            dbg_sp = {nm: dram.tile([128, S], F32, name="dbg_" + nm)
                      for nm in ("uga", "ugb", "vba", "vbb", "z1a", "z1b")} \
                if getattr(build, "DEBUG", False) else None
            cc_in = dram.tile([1, 1], F32)
            cc_out = dram.tile([1, 1], F32)

            gsum = pers.tile([1, NB * NGC_PER_BAND + 8], F32)
            nc.vector.memset(gsum[:], 0.0)
            dynk = pers.tile([96, 1], F32)
            probsT = pers.tile([96, 96], F32R)
            _load_w(pers, ["ident", "identr"])
            ident = ws["ident"]
            identr = ws["identr"]

            # ================= PHASE A =================
            _wpab_cm = tc.tile_pool(name="wpAB", bufs=1)
            wpab = _wpab_cm.__enter__()
            _load_w(wpab, [k for k in WSPEC if k not in C_ONLY_W and k != "ident"])
            with (
                tc.tile_pool(name="pa_band", bufs=2) as pab,
                tc.tile_pool(name="pa_rot", bufs=3) as par,
                tc.tile_pool(name="pa_ps", bufs=2, space="PSUM") as paps,
                tc.tile_pool(name="pa_ps2", bufs=2, space="PSUM") as paps2,
            ):
                for b in range(NB):
                    r0, r1 = b * BR, min((b + 1) * BR, H)
                    xband = [pab.tile([96, P1B], F32R, tag=f"xb{cg}", name=f"xb{cg}") for cg in range(2)]
                    for cg in range(2):
                        nc.vector.memset(xband[cg][:].bitcast(F32), 0.0)
                    for rr in range(max(r0 - 1, 0), min(r1 + 1, H)):
                        xrow = par.tile([W, EMBED], F32, tag="xrow")
                        nc.sync.dma_start(out=xrow[:], in_=x_t[rr])
                        boff = SLOP + (rr - (r0 - 1)) * Wp1 + 1
                        for cg in range(2):
                            tps = paps2.tile([96, W], F32, tag="tps")
                            nc.tensor.transpose(tps[:], xrow[:, cg * 96:(cg + 1) * 96],
                                                ident[:W, :W])
                            nc.scalar.copy(xband[cg][:, boff:boff + W], tps[:])
                    for c0 in range(r0, r1, RC):
                        nr_c = min(RC, H - c0)
                        N = nr_c * Wp1
                        NN = nr_c * W
                        sb0 = SLOP + (c0 - r0 + 1) * Wp1
                        xc_ch = [par.tile([96, RC * W], F32R, tag=f"xc{cg}", name=f"xc{cg}") for cg in range(2)]
                        xsq = [par.tile([96, RC * W], F32R, tag=f"xq{cg}", name=f"xq{cg}") for cg in range(2)]
                        for cg in range(2):
                            ps = paps.tile([96, RC * Wp1], F32, tag="posps")
                            for t in range(9):
                                dy, dx = t // 3 - 1, t % 3 - 1
                                o = sb0 + dy * Wp1 + dx
                                nc.tensor.matmul(
                                    ps[:, :N],
                                    ws["pos_diag"][:, (t * 2 + cg) * 96:(t * 2 + cg + 1) * 96],
                                    xband[cg][:, o:o + N],
                                    start=(t == 0), stop=(t == 8))
                            ps_int = ps[:, :N].rearrange("p (r w) -> p r w", w=Wp1)[:, :, 1:1 + W]
                            xb_int = xband[cg][:, sb0:sb0 + N] \
                                .rearrange("p (r w) -> p r w", w=Wp1)[:, :, 1:1 + W]
                            xcv = xc_ch[cg][:, :NN].rearrange("p (r w) -> p r w", w=W)
                            nc.vector.scalar_tensor_tensor(
                                out=xcv, in0=ps_int, scalar=ws["pos_b"][:, cg:cg + 1],
                                in1=xb_int, op0=Alu.add, op1=Alu.add)
                            nc.scalar.square(xsq[cg][:, :NN], xc_ch[cg][:, :NN])
                        mu_ps = paps.tile([128, RC * W], F32, tag="mups")
                        m2_ps = paps.tile([128, RC * W], F32, tag="m2ps")
                        for cg in range(2):
                            nc.tensor.matmul(mu_ps[:, :NN], ws["ones_st"], xc_ch[cg][:, :NN],
                                             start=(cg == 0), stop=(cg == 1))
                            nc.tensor.matmul(m2_ps[:, :NN], ws["ones_st"], xsq[cg][:, :NN],
                                             start=(cg == 0), stop=(cg == 1))
                        musq = par.tile([128, RC * W], F32, tag="musq")
                        nc.scalar.square(musq[:, :NN], mu_ps[:, :NN])
                        var = par.tile([128, RC * W], F32, tag="var")
                        nc.vector.tensor_tensor(out=var[:, :NN], in0=m2_ps[:, :NN],
                                                in1=musq[:, :NN], op=Alu.subtract)
                        sd = par.tile([128, RC * W], F32, tag="sd")
                        nc.scalar.activation(sd[:, :NN], var[:, :NN], Act.Sqrt, bias=ws["epsv"])
                        rstd = par.tile([128, RC * W], F32, tag="rstd")
                        nc.vector.reciprocal_approx_fast(rstd[:, :NN], sd[:, :NN])
                        for cg in range(2):
                            tdf = par.tile([96, RC * W], F32, tag=f"td{cg}")
                            nc.vector.tensor_tensor(out=tdf[:, :NN], in0=xc_ch[cg][:, :NN],
                                                    in1=mu_ps[:96, :NN], op=Alu.subtract)
                            ynch = par.tile([96, RC * W], F32R, tag=f"yn{cg}")
                            nc.vector.tensor_tensor(out=ynch[:, :NN], in0=tdf[:, :NN],
                                                    in1=rstd[:96, :NN], op=Alu.mult)
                            sp = yn1_sp if cg == 0 else yn2_sp
                            nc.sync.dma_start(out=sp[:, c0 * W:c0 * W + NN],
                                              in_=ynch[:, :NN])
                            nc.sync.dma_start(out=xc_sp[cg][:, c0 * W:c0 * W + NN],
                                              in_=xc_ch[cg][:, :NN])

            # ================= PHASE B =================
            with (
                tc.tile_pool(name="pb_band", bufs=2) as pbb,
                tc.tile_pool(name="pb_rot", bufs=3) as pbr,
                tc.tile_pool(name="gram_ps", bufs=1, space="PSUM") as gpsp,
            ):
                # one bank: [0:96]=q.q, [96:192]=q.k, [192:288]=k.k
                g1_ps = gpsp.tile([96, 288], F32)
                with (
                    tc.tile_pool(name="pb_psg", bufs=1, space="PSUM") as pbpsg,
                    tc.tile_pool(name="pb_ps", bufs=2, space="PSUM") as pbps,
                ):
                    for b in range(NB):
                        r0, r1 = b * BR, min((b + 1) * BR, H)
                        ylo, yhi = max(r0 - 1, 0), min(r1 + 1, H)
                        ynb = [pbb.tile([96, (BR + 2) * W], F32R, tag=f"ynb{cg}", name=f"ynb{cg}")
                               for cg in range(2)]
                        for cg in range(2):
                            sp = yn1_sp if cg == 0 else yn2_sp
                            nc.sync.dma_start(
                                out=ynb[cg][:, (ylo - r0 + 1) * W:(yhi - r0 + 1) * W],
                                in_=sp[:, ylo * W:yhi * W])
                        # gate (512-col chunks over rows [r0, r1))
                        for gi in range(NGC_PER_BAND):
                            gr0 = r0 + gi * GCH
                            if gr0 >= r1:
                                break
                            ngr = min(GCH, r1 - gr0)
                            NG = ngr * W
                            yo = (gr0 - r0 + 1) * W
                            gps = pbpsg.tile([96, 512], F32, tag="gps")
                            for cg in range(2):
                                nc.tensor.matmul(gps[:, :NG],
                                                 ws["gate_w1T"][:, cg * 96:(cg + 1) * 96],
                                                 ynb[cg][:, yo:yo + NG],
                                                 start=(cg == 0), stop=(cg == 1))
                            g1s = pbr.tile([96, 512], F32R, tag="g1s")
                            nc.scalar.activation(g1s[:, :NG], gps[:, :NG], Act.Relu,
                                                 bias=ws["gate_b1"])
                            g2ps = pbpsg.tile([1, 512], F32, tag="g2ps")
                            nc.tensor.matmul(g2ps[:, :NG], ws["gate_w2T"], g1s[:, :NG],
                                             start=True, stop=True)
                            sgt = pbr.tile([1, 512], F32, tag="sgt")
                            idx = b * NGC_PER_BAND + gi
                            nc.scalar.activation(sgt[:, :NG], g2ps[:, :NG], Act.Sigmoid,
                                                 bias=ws["gate_b2"],
                                                 accum_out=gsum[0:1, idx:idx + 1])
                        # qkv0 band
                        qkv0 = [pbb.tile([96, P1B], F32R, tag=f"qk0{j}", name=f"qk0{j}") for j in range(3)]
                        for j in range(3):
                            nc.vector.memset(qkv0[j][:].bitcast(F32), 0.0)
                        for rr in range(ylo, yhi, 2):
                            nrw = min(2, yhi - rr)
                            NQ = nrw * W
                            for j in range(3):
                                qps = pbps.tile([96, 2 * W], F32, tag="qps", bufs=1)
                                nc.tensor.matmul(qps[:, :NQ],
                                                 ws["qkv_wT"][:, j * 96:(j + 1) * 96],
                                                 ynb[0][:, (rr - r0 + 1) * W:(rr - r0 + 1) * W + NQ],
                                                 start=True, stop=True)
                                dst = SLOP + (rr - r0 + 1) * Wp1 + 1
                                dview = qkv0[j][:, dst:dst + nrw * Wp1] \
                                    .rearrange("p (r w) -> p r w", w=Wp1)[:, :, 0:W]
                                nc.scalar.activation(
                                    dview, qps[:, :NQ].rearrange("p (r w) -> p r w", w=W),
                                    Act.Identity, bias=ws["qkv_bias"][:, j:j + 1])
                        # depthwise, then per-row TensorE transpose + fp32 gram
                        for c0 in range(r0, r1, RC):
                            nr_c = min(RC, H - c0)
                            N = nr_c * Wp1
                            NN = nr_c * W
                            sb0 = SLOP + (c0 - r0 + 1) * Wp1
                            qk_ch = {}
                            for j in range(3):
                                ps = pbps.tile([96, RC * Wp1], F32, tag="dwps")
                                for t in range(9):
                                    dy, dx = t // 3 - 1, t % 3 - 1
                                    o = sb0 + dy * Wp1 + dx
                                    nc.tensor.matmul(
                                        ps[:, :N],
                                        ws["qdw_diag"][:, (t * 3 + j) * 96:(t * 3 + j + 1) * 96],
                                        qkv0[j][:, o:o + N],
                                        start=(t == 0), stop=(t == 8))
                                ps_int = ps[:, :N].rearrange("p (r w) -> p r w", w=Wp1)[:, :, 1:1 + W]
                                ch = pbr.tile([96, RC * W], F32R, tag=f"qkv_ch{j}")
                                nc.scalar.copy(
                                    ch[:, :NN].rearrange("p (r w) -> p r w", w=W), ps_int)
                                if j == 2:
                                    nc.sync.dma_start(out=v_sp[:, c0 * W:c0 * W + NN],
                                                      in_=ch[:, :NN])
                                else:
                                    qk_ch[j] = ch
                            for rr in range(c0, c0 + nr_c):
                                rl = (rr - c0) * W
                                tqk_ps = pbps.tile([W, 192], F32, tag="tqk")
                                nc.tensor.transpose(
                                    tqk_ps[:, 0:96],
                                    qk_ch[0][:, rl:rl + W].bitcast(F32),
                                    ident[:96, :96])
                                nc.tensor.transpose(
                                    tqk_ps[:, 96:192],
                                    qk_ch[1][:, rl:rl + W].bitcast(F32),
                                    ident[:96, :96])
                                qkT = pbr.tile([W, 192], F32, tag="qkT")
                                nc.scalar.copy(qkT[:], tqk_ps[:])
                                nc.tensor.matmul(g1_ps[:, 0:192], qkT[:, 0:96],
                                                 qkT[:, 0:192],
                                                 start=(rr == 0), stop=(rr == H - 1))
                                nc.tensor.matmul(g1_ps[:, 192:288], qkT[:, 96:192],
                                                 qkT[:, 96:192],
                                                 start=(rr == 0), stop=(rr == H - 1))

                # ---- gate mean -> AllReduce -> dynk ----
                gred = pers.tile([1, 1], F32)
                nc.vector.reduce_sum(gred[:], gsum[0:1, 0:NB * NGC_PER_BAND], axis=AX)
                gsc = pers.tile([1, 1], F32)
                nc.vector.tensor_scalar_mul(gsc[:], gred[:], float(CPH) / (n_cores * S))
                nc.sync.dma_start(out=cc_in[:], in_=gsc[:])
                nc.gpsimd.collective_compute(
                    "AllReduce", Alu.add, replica_groups=[list(range(n_cores))],
                    ins=[cc_in.opt()], outs=[cc_out.opt()])
                nc.sync.dma_start(out=dynk[:], in_=cc_out[:].partition_broadcast(96))

                # ---- attn block ----
                with (
                    tc.tile_pool(name="at_ps", bufs=2, space="PSUM") as atps,
                    tc.tile_pool(name="at_sb", bufs=1) as ab,
                ):
                    g1sb = ab.tile([96, 288], F32)
                    nc.scalar.copy(g1sb[:], g1_ps[:])
                    gqk = g1sb[:, 96:192]
                    idm = ident[:96, :96]
                    tq = ab.tile([96, 96], F32)
                    nc.vector.tensor_tensor(out=tq[:], in0=g1sb[:, 0:96], in1=idm, op=Alu.mult)
                    nq2 = ab.tile([96, 1], F32)
                    nc.vector.reduce_sum(nq2[:], tq[:], axis=AX)
                    tk = ab.tile([96, 96], F32)
                    nc.vector.tensor_tensor(out=tk[:], in0=g1sb[:, 192:288], in1=idm,
                                            op=Alu.mult)
                    nk2 = ab.tile([96, 1], F32)
                    nc.vector.reduce_sum(nk2[:], tk[:], axis=AX)

                    def rsqrt_clamped(nm, src):
                        sq = ab.tile([96, 1], F32, tag=nm + "sq")
                        nc.scalar.sqrt(sq[:], src[:])
                        cl = ab.tile([96, 1], F32, tag=nm + "cl")
                        nc.vector.tensor_scalar_max(cl[:], sq[:], 1e-12)
                        rvv = ab.tile([96, 1], F32, tag=nm)
                        nc.vector.reciprocal_approx_fast(rvv[:], cl[:])
                        return rvv

                    rq = rsqrt_clamped("rq", nq2)
                    rk = rsqrt_clamped("rk", nk2)
                    rqt = ab.tile([96, 1], F32)
                    nc.vector.tensor_tensor(out=rqt[:], in0=rq[:], in1=ws["tempvec"][:],
                                            op=Alu.mult)
                    asr = ab.tile([96, 96], F32)
                    nc.vector.tensor_scalar_mul(asr[:], gqk, rqt[:])
                    as_ps = atps.tile([96, 96], F32, tag="atp")
                    nc.tensor.transpose(as_ps[:], asr[:], ident[:96, :96])
                    ast = ab.tile([96, 96], F32)
                    nc.vector.tensor_scalar_mul(ast[:], as_ps[:], rk[:])
                    as2_ps = atps.tile([96, 96], F32, tag="atp")
                    nc.tensor.transpose(as2_ps[:], ast[:], ident[:96, :96])
                    as2 = ab.tile([96, 96], F32)
                    nc.scalar.copy(as2[:], as2_ps[:])
                    # mask off-head-block entries to -60
                    t60 = ab.tile([96, 96], F32)
                    nc.vector.tensor_scalar_add(t60[:], as2[:], 60.0)
                    amf = ab.tile([96, 96], F32)
                    nc.vector.tensor_tensor(out=amf[:], in0=t60[:], in1=ws["vmask"][:],
                                            op=Alu.mult)
                    nc.vector.tensor_scalar_add(amf[:], amf[:], -60.0)
                    # rank+1 over full row via pairwise is_ge
                    rnk3 = ab.tile([96, 96 * 96], F32)
                    a_i = amf[:].unsqueeze(1).broadcast_to([96, 96, 96])
                    a_d = amf[:].unsqueeze(2).broadcast_to([96, 96, 96])
                    rvw = rnk3[:].rearrange("p (i d) -> p i d", d=96)
                    nc.vector.tensor_tensor(out=rvw, in0=a_i, in1=a_d, op=Alu.is_ge)
                    rank1 = ab.tile([96, 96], F32)
                    nc.vector.reduce_sum(rank1[:].unsqueeze(2), rvw, axis=AX)
                    sel = ab.tile([96, 96], F32)
                    nc.vector.tensor_tensor(out=sel[:], in0=rank1[:],
                                            in1=dynk[:].broadcast_to([96, 96]), op=Alu.is_le)
                    am = ab.tile([96, 96], F32)
                    t60b = ab.tile([96, 96], F32)
                    nc.vector.tensor_scalar_add(t60b[:], amf[:], 60.0)
                    nc.vector.tensor_tensor(out=am[:], in0=t60b[:], in1=sel[:], op=Alu.mult)
                    nc.vector.tensor_scalar_add(am[:], am[:], -60.0)
                    mx = ab.tile([96, 1], F32)
                    nc.vector.reduce_max(mx[:], am[:], axis=AX)
                    nmx = ab.tile([96, 1], F32)
                    nc.vector.tensor_scalar_mul(nmx[:], mx[:], -1.0)
                    ex = ab.tile([96, 96], F32)
                    nc.scalar.activation(ex[:], am[:], Act.Exp, bias=nmx[:])
                    sme = ab.tile([96, 1], F32)
                    nc.vector.reduce_sum(sme[:], ex[:], axis=AX)
                    rsm = ab.tile([96, 1], F32)
                    nc.vector.reciprocal_approx_fast(rsm[:], sme[:])
                    probs = ab.tile([96, 96], F32)
                    nc.vector.tensor_scalar_mul(probs[:], ex[:], rsm[:])
                    pt_ps = atps.tile([96, 96], F32, tag="atp2")
                    nc.tensor.transpose(pt_ps[:], probs[:], ident[:96, :96])
                    nc.scalar.copy(probsT[:], pt_ps[:])

            # ================= PHASE B5 =================
            with (
                tc.tile_pool(name="b5_rot", bufs=3) as b5r,
                tc.tile_pool(name="b5_ps", bufs=1, space="PSUM") as b5ps,
            ):
                for ci in range(NSC):
                    o0 = ci * 512
                    NN = min(512, S - o0)
                    vch = b5r.tile([96, 512], F32R, tag="vch")
                    nc.sync.dma_start(out=vch[:, :NN], in_=v_sp[:, o0:o0 + NN])
                    av_ps = b5ps.tile([96, 512], F32, tag="avps", bufs=2)
                    nc.tensor.matmul(av_ps[:, :NN], probsT[:], vch[:, :NN],
                                     start=True, stop=True)
                    avs = b5r.tile([96, 512], F32R, tag="avs")
                    nc.scalar.activation(avs[:, :NN], av_ps[:, :NN], Act.Copy,
                                         scale=attn_scale)
                    x2ch = b5r.tile([96, 512], F32R, tag="x2ch")
                    nc.sync.dma_start(out=x2ch[:, :NN], in_=yn2_sp[:, o0:o0 + NN])
                    xpch = [b5r.tile([96, 512], F32R, tag=f"xp{cg}", name=f"xp{cg}") for cg in range(2)]
                    xsq = [b5r.tile([96, 512], F32R, tag=f"xs{cg}", name=f"xs{cg}") for cg in range(2)]
                    for cg in range(2):
                        xcch = b5r.tile([96, 512], F32R, tag=f"xcc{cg}")
                        nc.sync.dma_start(out=xcch[:, :NN], in_=xc_sp[cg][:, o0:o0 + NN])
                        pj_ps = b5ps.tile([96, 512], F32, tag=f"pjps{cg}", bufs=2)
                        nc.tensor.matmul(pj_ps[:, :NN],
                                         ws["proj1T"][:, cg * 96:(cg + 1) * 96],
                                         avs[:, :NN], start=True, stop=False)
                        nc.tensor.matmul(pj_ps[:, :NN],
                                         ws["proj2T"][:, cg * 96:(cg + 1) * 96],
                                         x2ch[:, :NN], start=False, stop=True)
                        nc.vector.scalar_tensor_tensor(
                            out=xpch[cg][:, :NN], in0=pj_ps[:, :NN],
                            scalar=ws["proj_bias"][:, cg:cg + 1], in1=xcch[:, :NN],
                            op0=Alu.add, op1=Alu.add)
                        nc.sync.dma_start(out=xcp_sp[cg][:, o0:o0 + NN],
                                          in_=xpch[cg][:, :NN])
                        nc.scalar.square(xsq[cg][:, :NN], xpch[cg][:, :NN])
                    # LN2 applied here; spill the normalized activations so the
                    # C-phase loop needs no stats broadcast at all
                    mu_ps = b5ps.tile([128, 512], F32, tag="mu2ps", bufs=1)
                    m2_ps = b5ps.tile([128, 512], F32, tag="m22ps", bufs=1)
                    for cg in range(2):
                        nc.tensor.matmul(mu_ps[:, :NN], ws["ones_st"], xpch[cg][:, :NN],
                                         start=(cg == 0), stop=(cg == 1))
                        nc.tensor.matmul(m2_ps[:, :NN], ws["ones_st"], xsq[cg][:, :NN],
                                         start=(cg == 0), stop=(cg == 1))
                    musq = b5r.tile([128, 512], F32, tag="musq2")
                    nc.scalar.square(musq[:, :NN], mu_ps[:, :NN])
                    var = b5r.tile([128, 512], F32, tag="var2")
                    nc.vector.tensor_tensor(out=var[:, :NN], in0=m2_ps[:, :NN],
                                            in1=musq[:, :NN], op=Alu.subtract)
                    sd = b5r.tile([128, 512], F32, tag="sd2")
                    nc.scalar.activation(sd[:, :NN], var[:, :NN], Act.Sqrt, bias=ws["epsv"])
                    rstd = b5r.tile([128, 512], F32, tag="rstd2")
                    nc.vector.reciprocal_approx_fast(rstd[:, :NN], sd[:, :NN])
                    for cg in range(2):
                        td = b5r.tile([96, 512], F32, tag=f"td{cg}")
                        nc.vector.tensor_tensor(out=td[:, :NN], in0=xpch[cg][:, :NN],
                                                in1=mu_ps[:96, :NN], op=Alu.subtract)
                        ynn = b5r.tile([96, 512], F32R, tag=f"ynn{cg}")
                        nc.vector.tensor_tensor(out=ynn[:, :NN], in0=td[:, :NN],
                                                in1=rstd[:96, :NN], op=Alu.mult)
                        nc.sync.dma_start(out=ynn_sp[cg][:, o0:o0 + NN],
                                          in_=ynn[:, :NN])

            _wpab_cm.__exit__(None, None, None)
            # ================= PHASE C =================
            _wpc_cm = tc.tile_pool(name="wpC", bufs=1)
            wpc = _wpc_cm.__enter__()
            _load_w(wpc, C_ONLY_W)
            with tc.tile_pool(name="c_v0", bufs=1) as cv0:
                v0t1 = cv0.tile([128, P3], F32R)
                v0t2 = cv0.tile([128, P3], F32R)
                with (
                    tc.tile_pool(name="c1_rot", bufs=2) as c1r,
                    tc.tile_pool(name="c2_rot", bufs=2) as c2r,
                    tc.tile_pool(name="c_ps", bufs=1, space="PSUM") as cps,
                ):
                    # pad cells must hold -t1/s1 so the bn-folded depthwise
                    # reads zeros in v0_bn space at image borders; only the
                    # border strips are ever read as pad (interior is written
                    # by the fc1 stage below), so skip the full-buffer memset
                    for v0t, pv in ((v0t1, ws["padv1"]), (v0t2, ws["padv2"])):
                        strips = [
                            v0t[:, 0:SLOP + 3 * Wp3],
                            v0t[:, SLOP + (H + 3) * Wp3:P3],
                        ]
                        mid = v0t[:, SLOP + 3 * Wp3:SLOP + (H + 3) * Wp3] \
                            .rearrange("p (r w) -> p r w", w=Wp3)
                        strips.append(mid[:, :, 0:4])
                        strips.append(mid[:, :, 131:134])
                        for st in strips:
                            nc.vector.memset(st.bitcast(F32), 0.0)
                            nc.vector.tensor_scalar_add(st, st, pv)

                    def c1_body(ci):
                        c0 = ci * RC
                        nr_c = min(RC, H - c0)
                        NN = nr_c * W
                        o0 = c0 * W
                        yn2t = [c1r.tile([96, RC * W], F32R, tag=f"cy{cg}", name=f"cy{cg}") for cg in range(2)]
                        for cg in range(2):
                            nc.sync.dma_start(out=yn2t[cg][:, :NN],
                                              in_=ynn_sp[cg][:, o0:o0 + NN])
                        sb0c = pd3(3 + c0)
                        # g0: channels 0:64 -> vg0/ug0
                        fg0 = cps.tile([64, RC * W], F32, tag="fg0", bufs=1)
                        for cg in range(2):
                            nc.tensor.matmul(fg0[:, :NN],
                                             ws["fc1T"][:, cg * 64:(cg + 1) * 64],
                                             yn2t[cg][:, :NN],
                                             start=(cg == 0), stop=(cg == 1))
                        fg2 = cps.tile([64, RC * W], F32, tag="fg2", bufs=1)
                        for cg in range(2):
                            nc.tensor.matmul(fg2[:, :NN],
                                             ws["fc1T"][:, 384 + cg * 64:448 + cg * 64],
                                             yn2t[cg][:, :NN],
                                             start=(cg == 0), stop=(cg == 1))
                        vg0 = c1r.tile([64, RC * W], F32R, tag="vg0")
                        nc.scalar.activation(vg0[:, :NN], fg0[:, :NN], Act.Gelu,
                                             bias=ws["fc1_bias"][0:64, 0:1])
                        ug0 = c1r.tile([64, RC * W], F32R, tag="ug0")
                        nc.scalar.activation(ug0[:, :NN], vg0[:, :NN], Act.Gelu,
                                             bias=ws["g0_bias"], scale=ws["g0_scale"])
                        for r in range(nr_c):
                            d0 = pd3(3 + c0 + r) + 3
                            nc.sync.dma_start(out=vg0_sp[:, d0:d0 + W],
                                              in_=vg0[:, r * W:(r + 1) * W])
                            nc.sync.dma_start(out=ug0_sp[:, d0:d0 + W],
                                              in_=ug0[:, r * W:(r + 1) * W])
                        # g1: channels 64:192 -> v0t1, one strided gelu
                        fg1 = cps.tile([128, RC * W], F32, tag="fg1", bufs=1)
                        for cg in range(2):
                            nc.tensor.matmul(fg1[:, :NN],
                                             ws["fc1T"][:, 128 + cg * 128:256 + cg * 128],
                                             yn2t[cg][:, :NN],
                                             start=(cg == 0), stop=(cg == 1))
                        dv1 = v0t1[:, sb0c:sb0c + nr_c * Wp3].rearrange(
                            "p (r w) -> p r w", w=Wp3)[:, :, 3:3 + W]
                        nc.scalar.activation(
                            dv1, fg1[:, :NN].rearrange("p (r w) -> p r w", w=W),
                            Act.Gelu, bias=ws["fc1_bias"][:, 1:2])
                        # g2: channels 192:256 -> v0t2 halves (second shifted +1)
                        dv2a = v0t2[0:64, sb0c:sb0c + nr_c * Wp3].rearrange(
                            "p (r w) -> p r w", w=Wp3)[:, :, 3:3 + W]
                        nc.scalar.activation(
                            dv2a, fg2[:, :NN].rearrange("p (r w) -> p r w", w=W),
                            Act.Gelu, bias=ws["fc1_bias"][64:128, 0:1])
                        dv2b = v0t2[64:128, sb0c:sb0c + nr_c * Wp3].rearrange(
                            "p (r w) -> p r w", w=Wp3)[:, :, 4:4 + W]
                        nc.scalar.activation(
                            dv2b, fg2[:, :NN].rearrange("p (r w) -> p r w", w=W),
                            Act.Gelu, bias=ws["fc1_bias"][64:128, 0:1])

                    def c2_body(ci):
                        c0 = ci * RC
                        nr_c = min(RC, H - c0)
                        N = nr_c * Wp3
                        NN = nr_c * W
                        sb0 = pd3(3 + c0)
                        ps_a = cps.tile([128, RC * Wp3], F32, tag="psa", bufs=2)
                        for t in range(25):
                            dy, dx = t // 5 - 2, t % 5 - 2
                            o = sb0 + dy * Wp3 + dx
                            nc.tensor.matmul(ps_a[:, :N],
                                             ws["pair_diag"][:, t * 128:(t + 1) * 128],
                                             v0t1[:, o:o + N],
                                             start=(t == 0), stop=(t == 24))
                        ps_b = cps.tile([64, RC * Wp3], F32, tag="psb", bufs=1)
                        for i, (dy, dxa, hasb) in enumerate(dw3_passes):
                            o = sb0 + dy * Wp3 + dxa
                            nc.tensor.matmul(ps_b[:, :N],
                                             ws["dw3_diag"][:, i * 64:(i + 1) * 64],
                                             v0t2[:, o:o + N],
                                             start=(i == 0), stop=(i == len(dw3_passes) - 1))

                        def inner(ap_flat, lo, hi):
                            # interior view of a PSUM chunk (starts at free 0)
                            return ap_flat[lo:hi, :N].rearrange(
                                "p (r w) -> p r w", w=Wp3)[:, :, 3:3 + W]

                        def inner_v0(ap_flat, lo, hi):
                            # interior view of the padded v0 buffers at this chunk
                            return ap_flat[lo:hi, sb0:sb0 + N].rearrange(
                                "p (r w) -> p r w", w=Wp3)[:, :, 3:3 + W]

                        ug_a = c2r.tile([128, RC * W], F32R, tag="uga")
                        ug_b = c2r.tile([128, RC * W], F32R, tag="ugb")
                        vb_a = c2r.tile([128, RC * W], F32R, tag="vba")
                        vb_b = c2r.tile([128, RC * W], F32R, tag="vbb")
                        g0v = c2r.tile([64, RC * W], F32R, tag="g0v")
                        src3 = vg0_sp[:, sb0:sb0 + N].rearrange(
                            "p (r w) -> p r w", w=Wp3)[:, :, 3:3 + W]
                        nc.sync.dma_start(
                            out=g0v[:, :NN].rearrange("p (r w) -> p r w", w=W), in_=src3)
                        usrc3 = ug0_sp[:, sb0:sb0 + N].rearrange(
                            "p (r w) -> p r w", w=Wp3)[:, :, 3:3 + W]
                        nc.sync.dma_start(
                            out=ug_a[0:64, :NN].rearrange("p (r w) -> p r w", w=W), in_=usrc3)
                        nc.scalar.activation(
                            ug_a[64:128, :NN].rearrange("p (r w) -> p r w", w=W),
                            inner(ps_a, 0, 64), Act.Gelu, bias=ws["pair_bias"][0:64])
                        nc.scalar.activation(
                            ug_b[0:64, :NN].rearrange("p (r w) -> p r w", w=W),
                            inner(ps_a, 64, 128), Act.Gelu, bias=ws["pair_bias"][64:128])
                        nc.scalar.activation(
                            ug_b[64:128, :NN].rearrange("p (r w) -> p r w", w=W),
                            inner(ps_b, 0, 64), Act.Gelu, bias=ws["dw3_bias"])
                        nc.vector.tensor_scalar(out=vb_a[0:64, :NN], in0=g0v[:, :NN],
                                                scalar1=ws["s1a"][0:64],
                                                scalar2=ws["t1a"][0:64],
                                                op0=Alu.mult, op1=Alu.add)
                        nc.vector.tensor_scalar(out=vb_a[64:128, :NN],
                                                in0=inner_v0(v0t1, 0, 64),
                                                scalar1=ws["s1a"][64:128],
                                                scalar2=ws["t1a"][64:128],
                                                op0=Alu.mult, op1=Alu.add)
                        nc.vector.tensor_scalar(out=vb_b[0:64, :NN],
                                                in0=inner_v0(v0t1, 64, 128),
                                                scalar1=ws["s1b"][0:64],
                                                scalar2=ws["t1b"][0:64],
                                                op0=Alu.mult, op1=Alu.add)
                        nc.vector.tensor_scalar(out=vb_b[64:128, :NN],
                                                in0=inner_v0(v0t2, 0, 64),
                                                scalar1=ws["s1b"][64:128],
                                                scalar2=ws["t1b"][64:128],
                                                op0=Alu.mult, op1=Alu.add)
                        # z1 = ug * vb computed in place into the vb tiles
                        nc.vector.tensor_tensor(out=vb_a[:, :NN], in0=ug_a[:, :NN],
                                                in1=vb_a[:, :NN], op=Alu.mult)
                        nc.vector.tensor_tensor(out=vb_b[:, :NN], in0=ug_b[:, :NN],
                                                in1=vb_b[:, :NN], op=Alu.mult)
                        occ = {}
                        tpps = {}
                        for cg in range(2):
                            # ops [96, 0:384] and the output-transpose psum
                            # [128, 384:480] share one bank
                            cmb = cps.tile([128, RC * W + 96], F32, tag=f"cmb{cg}",
                                           bufs=1)
                            ops = cmb[0:96, 0:RC * W]
                            ops = cmb[0:96, 0:NN]
                            tpps[cg] = cmb[:, RC * W:RC * W + 96]
                            nc.tensor.matmul(ops,
                                             ws["fc2aT"][:, (cg * 2) * 96:(cg * 2 + 1) * 96],
                                             vb_a[:, :NN], start=True, stop=False)
                            nc.tensor.matmul(ops,
                                             ws["fc2aT"][:, (cg * 2 + 1) * 96:(cg * 2 + 2) * 96],
                                             vb_b[:, :NN], start=False, stop=False)
                            nc.tensor.matmul(ops,
                                             ws["fc2bT_g0"][:, cg * 96:(cg + 1) * 96],
                                             g0v[:, :NN], start=False, stop=False)
                            opsv = ops.rearrange("p (r w) -> p r w", w=W)
                            nc.tensor.matmul(opsv,
                                             ws["fc2bT_g12"][:, cg * 96:(cg + 1) * 96],
                                             inner_v0(v0t1, 0, 128), start=False, stop=False)
                            nc.tensor.matmul(opsv,
                                             ws["fc2bT_g3"][:, cg * 96:(cg + 1) * 96],
                                             inner_v0(v0t2, 0, 64), start=False, stop=True)
                            xrch = c2r.tile([96, RC * W], F32R, tag=f"xr{cg}", bufs=1)
                            nc.sync.dma_start(out=xrch[:, :NN],
                                              in_=xcp_sp[cg][:, c0 * W:c0 * W + NN])
                            ob = c2r.tile([96, RC * W], F32, tag=f"ob{cg}", bufs=1)
                            nc.vector.tensor_scalar(out=ob[:, :NN], in0=ops,
                                                    scalar1=ws["s3v"][:, cg:cg + 1],
                                                    scalar2=ws["out_bias"][:, cg:cg + 1],
                                                    op0=Alu.mult, op1=Alu.add)
                            nc.vector.tensor_tensor(out=ob[:, :NN], in0=ob[:, :NN],
                                                    in1=xrch[:, :NN], op=Alu.add)
                            occ[cg] = ob
                        # transpose to pixel-major [W, EMBED] per image row so the
                        # output DMA writes contiguous 768B lines instead of a
                        # 4B-per-element scatter
                        outT = c2r.tile([W, RC * EMBED], F32, tag="outT", bufs=1)
                        for r in range(nr_c):
                            for cg in range(2):
                                nc.tensor.transpose(
                                    tpps[cg], occ[cg][:, r * W:(r + 1) * W],
                                    ws["ident"][:96, :96])
                                nc.scalar.copy(
                                    outT[:, r * EMBED + cg * 96:r * EMBED + (cg + 1) * 96],
                                    tpps[cg])
                        for r in range(nr_c):
                            nc.sync.dma_start(
                                out=out_t[(c0 + r) * W:(c0 + r + 1) * W, :],
                                in_=outT[:, r * EMBED:(r + 1) * EMBED])

                    # interleave: fc1/gelu of chunk it overlaps the
                    # TensorE-bound depthwise/fc2 of chunk it-2
                    for it in range(NCH + 2):
                        if it < NCH:
                            c1_body(it)
                        if it >= 2:
                            c2_body(it - 2)
            _wpc_cm.__exit__(None, None, None)
    return out_t.name


# ----------------------------------------------------------------------------
# host entry
# ----------------------------------------------------------------------------

_CACHE = {}


def make_program(H, W, n_cores, attn_scale, dw3_passes):
    key = (H, W, n_cores, round(attn_scale, 9))
    if key in _CACHE:
        return _CACHE[key]
    nc = bacc.Bacc("TRN2", target_bir_lowering=False, debug=False, num_devices=n_cores)
    out_name = build(nc, H, W, n_cores, attn_scale, dw3_passes)
    nc.compile()
    _CACHE[key] = (nc, out_name)
    return nc, out_name


def make_in_maps(inputs):
    x = np.asarray(inputs["x"], np.float32)
    B = x.shape[0]
    wdict = _prep_weights({k: np.asarray(v) for k, v in inputs.items()})
    base = {}
    for k, (shp, d) in WSPEC.items():
        base["w_" + k] = wdict[k][0].reshape(shp)
    in_maps = []
    for b in range(B):
        m = dict(base)
        m["x"] = np.ascontiguousarray(x[b])
        in_maps.append(m)
    return in_maps, wdict


def kernel(**inputs):
    x = np.asarray(inputs["x"], np.float32)
    B, H, W, C = x.shape
    in_maps, wdict = make_in_maps(inputs)
    nc, out_name = make_program(H, W, B, wdict["_attn_scale"][0],
                                wdict["_dw3_passes"][0])
    res = bass_utils.run_bass_kernel_spmd(nc, in_maps, core_ids=list(range(B)))
    return np.stack([res.results[b][out_name].reshape(H, W, C) for b in range(B)])

